# revision 29
# baseline (speedup 1.0000x reference)
"""Trainium2 Bass kernel for per-neuron MLPs (dense_mlp).

reference: out[b,d] = W2[d]^T.gelu(W1[d]^T.gelu(W0[d]^T.x[b,d,:]+b0)+b1)+b2
Shapes: x [256,2048,32], W0 [2048,32,64], W1 [2048,64,64], W2 [2048,64,1].

Sharding: D split across 8 cores (256 neurons each, fully independent).

Quadratic fast path (gated by _quad_ok): for this problem both hidden
pre-activations are tiny (|z0| < 0.1, |z1| < 5e-3), so
  gelu(z1) ~= z1/2          (collapses L1+gelu1+L2 into veff = W1@W2/2)
  gelu(z0) ~= z0/2 + c z0^2 (c = 1/sqrt(2pi); quartic term ~1e-5 rel)
and each neuron's whole MLP becomes
  out_d(x) = weff_d.x + sum_k s_k (g_k.x)^2
where Q_d = c W0 diag(veff) W0^T (32x32) = V diag(lam) V^T (host eigh),
g_k = sqrt|lam_k| v_k (fp8, per-neuron pow2 scale), s_k = sign(lam_k)
(carried as +-pow2 compensation in fp8), weff = W0.veff/2 (fp16).
This halves PE projections (32/neuron, not 64) and replaces the gelu LUT
stage with one cheap square per PSUM bank.  End-to-end rel err ~1.4e-3
vs the 2e-2 gate; the older lin/full pipelines remain as fallbacks.

Per-core dataflow (unit = 8 neurons = 2 quads, software-pipelined:
step t emits proj(t) | square(t-1) | reduce(t-4, pairs of units)):
  DMA: 17 chunks on BOTH HWDGE queues in consumption order - early-x on
      Scalar's queue, weights + late-x need-ordered on Sync's.  5.75MB
      total per core (x fp16 4MB, g/sgn fp8 1.25MB, weff fp16 0.5MB).
  proj: per quad one full-array matmul: block-diag g lhsT [128,128] fp8
      (rows 32q+m, cols 32q+k) x x-quad-stack [128,256] fp16 -> one PSUM
      bank zz [128,512] per unit (two quads side by side).
  square: whole units alternate ScalarE (Square LUT, scale 1/B) and DVE
      (custom C0*u^2 op) -> sq [128,512] fp16; one op per unit because
      the ~400ns fixed PSUM-access overhead dominates op size.
  reduce: per quad TWO 32-col-strip matmuls at tile_position (0,32(j%4))
      accumulate into l2[j//32] [128,512]: sgn strip (+-comp pow2) x sq
      gives the quadratic term; weff strip x the same x tile gives the
      linear term.  First writer per strip uses start=True (no memset).
      Batches of 4 quads keep all 4 column strips concurrently busy.
  evac: half 0 (quads 0-31) streams out at t=21 overlapping compute;
      half 1 after the loop.  o2 = l2 * (1/S_out) on ScalarE (+b2).
  Host re-stitches out[32(j%4)+4slot+q, 256hb+t] -> y[B, ND].
"""

import os
import sys

for _p in ("/opt/trn_rl_repo",):
    if _p not in sys.path:
        sys.path.insert(0, _p)

import numpy as np

import concourse.dve_ops as _dvo
from concourse import bacc, mybir, tile
from concourse import bass_utils as _bu
from concourse.bass_utils import run_bass_kernel_spmd


from concourse.dve_ops import DveOp, DveOpSpec, has_src1, lower as _dve_lower
from concourse.dve_spec import Spec, Src0, C0, C1, C2, One, sq

B = 256
D = 2048
M = 32
H = 64
NCORES = 8
ND = D // NCORES          # neurons per core = 256
NPAIR = ND // 2           # 128
NUNIT = ND // 8           # 32 units of 8 neurons (4 pairs)
GELU_C = 0.3989422804014327  # 1/sqrt(2*pi)
S_H1 = float(2 ** 14)     # fp16 scale for h1 (values ~1e-4 -> ~1.6)
S_V = float(2 ** 9)       # fp16 scale for veff = W1@W2/2 (values ~3e-5)

_f32 = mybir.dt.float32
_f16 = mybir.dt.float16


def _zc(c):
    """z0/h0 column of pair-in-unit c; concurrent row groups (c%2) get
    different PSUM banks."""
    return 512 * (c % 2) + 256 * (c // 2)


def _l2slot(p):
    """pair p -> (strip j, col half hb, partition slot m) in l2ps."""
    return p % 4, (p // 4) % 2, p // 8


_CH = [(0, 1), (1, 1), (2, 2), (4, 4), (8, 8), (16, 8), (24, 8)]
X_CHUNKS = list(_CH)
W_CHUNKS = list(_CH)


def _chunk_map(chunks):
    m = {}
    for k, (s, L) in enumerate(chunks):
        for u in range(s, s + L):
            m[u] = (k, u - s)
    return m


_XMAP = _chunk_map(X_CHUNKS)
_WMAP = _chunk_map(W_CHUNKS)


def _register_gelu_op():
    """out = u*(C1 + u*C0*(1 + u^2*C2)); with C0=S*c, C1=S/2, C2=-1/6 this is
    S*gelu(u) up to O(u^6) of the exact erf-gelu Taylor series."""
    name = "GELU_SCALED_ANT"
    for op in _dvo.OPS:
        if op.name == name:
            return op
    u = Src0
    body = u * (C1 + u * C0 * (One + sq(u) * C2))
    spec = Spec(
        body=body,
        reference=lambda in0, s0, s1, imm2: in0
        * (s1 + in0 * s0 * (1.0 + (in0 * in0) * imm2)),
    )
    shas = {}
    op = DveOp(name, spec, subdim=False, uops_sha=shas)
    _dvo.OPS.append(op)
    _dvo.CUSTOM_DVE_SPECS[name] = spec
    _dvo._SUB_OPCODE_FOR_NAME[name] = _dvo._CUSTOM_DVE_ROW_BASE + len(_dvo.OPS) - 1
    for ver in ("v3", "v4"):
        tmp = DveOpSpec(
            name=name,
            opcode=_dvo.get_dve_sub_opcode(name),
            uops=_dve_lower(spec, ver=ver),
            rd1_en=has_src1(spec),
        )
        shas[ver] = tmp.sha(ver)
    return op


_GELU_OP = _register_gelu_op()


def _register_sq_op():
    """out = C0 * Src0^2 — scaled square for the quadratic-gelu path."""
    name = "SQSCALE_ANT"
    for op in _dvo.OPS:
        if op.name == name:
            return op
    body = sq(Src0) * C0
    spec = Spec(
        body=body,
        reference=lambda in0, s0, s1, imm2: in0 * in0 * s0,
    )
    shas = {}
    op = DveOp(name, spec, subdim=False, uops_sha=shas)
    _dvo.OPS.append(op)
    _dvo.CUSTOM_DVE_SPECS[name] = spec
    _dvo._SUB_OPCODE_FOR_NAME[name] = _dvo._CUSTOM_DVE_ROW_BASE + len(_dvo.OPS) - 1
    for ver in ("v3", "v4"):
        tmp = DveOpSpec(
            name=name,
            opcode=_dvo.get_dve_sub_opcode(name),
            uops=_dve_lower(spec, ver=ver),
            rd1_en=has_src1(spec),
        )
        shas[ver] = tmp.sha(ver)
    return op


_SQ_OP = _register_sq_op()

_PROGRAM_CACHE = {}


def _build_program(use_b0, use_b1, use_b2, use_lin=False):
    ncores = int(os.environ.get("K_NCORES", NCORES))
    nrep = int(os.environ.get("K_NREP", 1))
    nc = bacc.Bacc("TRN2", target_bir_lowering=False, debug=False,
                   num_devices=ncores)

    ucols = 384 if use_lin else 640
    # x pair-stacks: xp[32q+m, 256j+t] = x[t, 4j+q, m]
    xp_d = nc.declare_dram_parameter("xp", [128, 64 * 256], _f16,
                                     isOutput=False)
    # all weights packed per unit.
    # full path (640 cols/unit: w0 256 | w1 256 | w2 128):
    #   w0 block: [64a+32b+m, 128*(j-2u)+64b+h] = W0[4j+2a+b][m,h]
    #   w1 block: [64b+h, 64c+o] = W1[2(4u+c)+b][h,o]
    #   w2 block: zero-padded blockdiag [64e+h, 32c+2m+e] = W2[2(4u+c)+e][h]
    # linearized path (384 cols/unit: w0 256 | veff 128), where
    #   veff[d] = S_V * (W1[d] @ W2[d]) / 2 replaces w1/w2 blocks.
    wall_d = nc.declare_dram_parameter("wall", [128, NUNIT * ucols], _f16,
                                       isOutput=False)
    if use_b2:
        b2_d = nc.declare_dram_parameter("b2bc", [128, 512], _f32,
                                         isOutput=False)
    if use_b0:
        # b0p[64b+h, p] = b0[2p+b][h]
        b0_d = nc.declare_dram_parameter("b0p", [128, NPAIR], _f32,
                                         isOutput=False)
    if use_b1:
        b1_d = nc.declare_dram_parameter("b1p", [128, NPAIR], _f32,
                                         isOutput=False)
    # out[32j+2m+e, 256hb+t] = y[t, 16m+8hb+2j+e]
    out_d = nc.declare_dram_parameter("out", [128, 512], _f32, isOutput=True)

    GELU = mybir.ActivationFunctionType.Gelu

    with tile.TileContext(nc) as tc:
        with (
            tc.tile_pool(name="wpool", bufs=1) as wpool,
            tc.tile_pool(name="xpool", bufs=1) as xpool,
            tc.tile_pool(name="h0pool", bufs=3) as h0pool,
            tc.tile_pool(name="h1pool", bufs=3) as h1pool,
            tc.tile_pool(name="opool", bufs=1) as opool,
            tc.tile_pool(name="psab", bufs=3, space="PSUM") as psab,
            tc.tile_pool(name="ps2", bufs=1, space="PSUM") as ps2,
        ):
            # Geometric unit-granular chunks; x chunks issue on the Sync
            # HWDGE queue, weight chunks on the Scalar HWDGE queue so the
            # two streams transfer concurrently and each queue only pays
            # ~650ns issue cost per chunk (7 chunks/queue, not 33 on one).
            xts = []
            wts = []
            deferred = []

            for i in range(max(len(X_CHUNKS), len(W_CHUNKS))):
                if i < len(X_CHUNKS):
                    s, L = X_CHUNKS[i]
                    xt = xpool.tile([128, L * 512], _f16, name="xt",
                                    tag=f"xt{i}")
                    nc.sync.dma_start(out=xt[:], in_=xp_d[:, s * 512:(s + L) * 512])
                    xts.append(xt)
                if i < len(W_CHUNKS):
                    s, L = W_CHUNKS[i]
                    wt = wpool.tile([128, L * ucols], _f16, name="wt",
                                    tag=f"wt{i}")
                    nc.scalar.dma_start(out=wt[:],
                                        in_=wall_d[:, s * ucols:(s + L) * ucols])
                    wts.append(wt)
            b0sb = b1sb = b2sb = None
            if use_b2:
                b2sb = wpool.tile([128, 512], _f32, tag="b2sb")
                nc.sync.dma_start(out=b2sb[:], in_=b2_d[:])
            if use_b0:
                b0sb = wpool.tile([128, NPAIR], _f32, tag="b0sb")
                nc.sync.dma_start(out=b0sb[:], in_=b0_d[:])
            if use_b1:
                b1sb = wpool.tile([128, NPAIR], _f32, tag="b1sb")
                nc.sync.dma_start(out=b1sb[:], in_=b1_d[:])

            for _rep in range(nrep):
                if use_lin:
                    _emit_body_lin(nc, h0pool, opool, psab, ps2,
                                   out_d, xts, wts, b0sb, b2sb, GELU,
                                   deferred)
                else:
                    _emit_body(nc, h0pool, h1pool, opool, psab, ps2,
                               out_d, xts, wts, b0sb, b1sb, b2sb, GELU)

    nc.finalize()
    return nc


def _emit_body_lin(nc, h0pool, opool, psab, ps2,
                   out_d, xts, wts, b0sb, b2sb, GELU, deferred=()):
    """gelu(z1) ~= z1/2 for |z1| << 1, so L1+gelu1+L2 collapse into one
    per-neuron vector veff = W1 @ W2 / 2 applied to h0 with the same
    zero-padded block-diag accumulate as L2."""
    l2ps = ps2.tile([128, 512], _f32, tag="l2")
    nc.vector.memset(l2ps[:], 0.0)

    z0 = {}
    h0 = {}

    def emit_l0(u):
        xk, xl = _XMAP[u]
        wk, wl = _WMAP[u]
        xt = xts[xk]
        wt = wts[wk]
        z0[u] = psab.tile([128, 1024], _f32, name="z0", tag="zz")
        for c in range(4):
            a = c % 2
            wcol = 384 * wl + 128 * (c // 2)
            xcol = 512 * xl + 256 * (c // 2)
            nc.tensor.matmul(
                z0[u][:, _zc(c):_zc(c) + 256],
                wt[64 * a:64 * a + 64, wcol:wcol + 128],
                xt[64 * a:64 * a + 64, xcol:xcol + 256],
                start=True, stop=True,
                tile_position=(64 * a, 0),
            )

    def emit_gelu0(u):
        # split across engines: ScalarE bank A (cols 0-511), DVE custom
        # poly bank B (cols 512-1023) -- parallel PSUM access, balanced
        # ~720ns vs ~680ns.
        h0[u] = h0pool.tile([128, 1024], _f16, name="h0", tag="h0")
        if b0sb is not None:
            for c in range(4):            # correct fallback: all-ScalarE
                p = 4 * u + c
                nc.scalar.activation(
                    h0[u][:, _zc(c):_zc(c) + 256],
                    z0[u][:, _zc(c):_zc(c) + 256],
                    GELU, bias=b0sb[:, p:p + 1], scale=1.0)
        else:
            nc.scalar.activation(h0[u][:, 0:512], z0[u][:, 0:512], GELU)
            nc.vector._custom_dve(
                _GELU_OP, out=h0[u][:, 512:1024], in0=z0[u][:, 512:1024],
                s0=GELU_C, s1=0.5, imm2=-1.0 / 6.0)
        del z0[u]

    def emit_l12(u):
        wk, wl = _WMAP[u]
        for c in range(4):
            p = 4 * u + c
            j, hb, m_ = _l2slot(p)
            wcol = 384 * wl + 256 + 32 * c
            nc.tensor.matmul(
                l2ps[32 * j:32 * j + 32, 256 * hb:256 * hb + 256],
                wts[wk][:, wcol:wcol + 32],
                h0[u][:, _zc(c):_zc(c) + 256],
                start=False, stop=False,
                tile_position=(0, 32 * j),
                skip_group_check=True,
            )
        del h0[u]

    for t in range(NUNIT + 2):
        if t < NUNIT:
            emit_l0(t)
        if 0 <= t - 1 < NUNIT:
            emit_gelu0(t - 1)
            for g, tile_, dram, lo, hi in deferred:
                if g == t:
                    # tiny GpSimd write orders the DMA (WAW) behind
                    # pipeline progress, keeping its transfer out of the
                    # startup ramp's bandwidth window
                    nc.gpsimd.tensor_copy(tile_[0:1, 0:2],
                                          h0[t - 1][0:1, 0:2])
                    nc.sync.dma_start(out=tile_[:], in_=dram[:, lo:hi])
        if 0 <= t - 2 < NUNIT:
            emit_l12(t - 2)

    o2 = opool.tile([128, 512], _f32, tag="o2")
    nc.scalar.mul(o2[:], l2ps[:], 1.0 / S_V)
    if b2sb is not None:
        nc.vector.tensor_add(o2[:], o2[:], b2sb[:])
    nc.sync.dma_start(out=out_d[:], in_=o2[:])


def _emit_body(nc, h0pool, h1pool, opool, psab, ps2,
               out_d, xts, wts, b0sb, b1sb, b2sb, GELU):
    l2ps = ps2.tile([128, 512], _f32, tag="l2")
    # Data is zeroed up front so every L2 matmul can use start=False:
    # first-writer overwrite and accumulate both produce 0 + v.
    nc.vector.memset(l2ps[:], 0.0)

    z0 = {}
    h0 = {}
    h1 = {}

    def emit_l0(u):
        xk, xl = _XMAP[u]
        wk, wl = _WMAP[u]
        xt = xts[xk]
        wt = wts[wk]
        z0[u] = psab.tile([128, 1024], _f32, name="z0", tag="zz")
        for c in range(4):
            a = c % 2
            wcol = 640 * wl + 128 * (c // 2)
            xcol = 512 * xl + 256 * (c // 2)
            nc.tensor.matmul(
                z0[u][:, _zc(c):_zc(c) + 256],
                wt[64 * a:64 * a + 64, wcol:wcol + 128],
                xt[64 * a:64 * a + 64, xcol:xcol + 256],
                start=True, stop=True,
                tile_position=(64 * a, 0),
            )

    def emit_gelu0(u):
        h0[u] = h0pool.tile([128, 1024], _f16, name="h0", tag="h0")
        if b0sb is not None:
            for c in range(4):
                p = 4 * u + c
                nc.scalar.activation(
                    h0[u][:, _zc(c):_zc(c) + 256],
                    z0[u][:, _zc(c):_zc(c) + 256],
                    GELU, bias=b0sb[:, p:p + 1], scale=1.0)
        else:
            nc.scalar.activation(h0[u][:], z0[u][:], GELU)
        del z0[u]

    def emit_l1_gelu1(u):
        z1 = psab.tile([128, 1024], _f32, name="z1", tag="zz")
        for c in range(4):
            p = 4 * u + c
            for b in range(2):
                rp = 64 * b
                wk, wl = _WMAP[u]
                nc.tensor.matmul(
                    z1[rp:rp + 64, 256 * c:256 * c + 256],
                    wts[wk][rp:rp + 64,
                            640 * wl + 256 + 64 * c:640 * wl + 320 + 64 * c],
                    h0[u][rp:rp + 64, _zc(c):_zc(c) + 256],
                    start=True, stop=True,
                    tile_position=(rp, rp),
                )
        gelu_in = z1
        if b1sb is not None:
            tmp = h0pool.tile([128, 1024], _f32, name="b1tmp", tag="b1tmp")
            for c in range(4):
                p = 4 * u + c
                nc.vector.tensor_scalar_add(
                    tmp[:, 256 * c:256 * c + 256],
                    z1[:, 256 * c:256 * c + 256],
                    b1sb[:, p:p + 1])
            gelu_in = tmp
        h1[u] = h1pool.tile([128, 1024], _f16, name="h1", tag="h1")
        nc.vector._custom_dve(
            _GELU_OP, out=h1[u][:], in0=gelu_in[:],
            s0=S_H1 * GELU_C, s1=S_H1 * 0.5, imm2=-1.0 / 6.0)
        del h0[u]

    def emit_l2(u):
        for c in range(4):
            p = 4 * u + c
            j, hb, m_ = _l2slot(p)
            ht = h1[u]
            wk, wl = _WMAP[u]
            wcol = 640 * wl + 512 + 32 * c
            nc.tensor.matmul(
                l2ps[32 * j:32 * j + 32, 256 * hb:256 * hb + 256],
                wts[wk][:, wcol:wcol + 32],
                ht[:, 256 * c:256 * c + 256],
                start=False, stop=False,
                tile_position=(0, 32 * j),
                skip_group_check=True,
            )
        del h1[u]

    for t in range(NUNIT + 3):
        if t < NUNIT:
            emit_l0(t)
        if 0 <= t - 1 < NUNIT:
            emit_gelu0(t - 1)
            emit_l1_gelu1(t - 1)
        if 0 <= t - 3 < NUNIT:
            emit_l2(t - 3)

    # ---- evac + store ----
    o2 = opool.tile([128, 512], _f32, tag="o2")
    nc.scalar.mul(o2[:], l2ps[:], 1.0 / S_H1)
    if b2sb is not None:
        nc.vector.tensor_add(o2[:], o2[:], b2sb[:])
    nc.sync.dma_start(out=out_d[:], in_=o2[:])


XQ_CHUNKS = [(0, 1), (1, 1), (2, 3), (5, 3), (8, 4), (12, 4), (16, 4),
             (20, 4), (24, 4), (28, 4)]
W8_CHUNKS = [(0, 1), (1, 4), (5, 8), (13, 9), (22, 10)]
WF_CHUNKS = [(0, 8), (8, 12), (20, 12)]
_XQMAP = _chunk_map(XQ_CHUNKS)
_W8MAP = _chunk_map(W8_CHUNKS)
_WFMAP = _chunk_map(WF_CHUNKS)
# x unit-blocks alternate between the two HWDGE queues so the x stream
# (70% of the bytes) rides ~full aggregate bandwidth in consumption
# order; weights fill the gaps on the Sync queue.  Entries: (queue,
# kind, idx), emitted in global first-need order.
_DMA_ORDER = [("B", "w8", 0), ("A", "x", 0), ("B", "x", 1), ("B", "w8", 1),
              ("A", "x", 2), ("B", "x", 3), ("B", "w8", 2), ("A", "wf", 0),
              ("A", "x", 4), ("B", "x", 5), ("B", "w8", 3), ("A", "wf", 1),
              ("A", "x", 6), ("B", "x", 7), ("B", "w8", 4), ("A", "wf", 2),
              ("A", "x", 8), ("B", "x", 9)]
_f8 = mybir.dt.float8e4


def _quad_slot(j):
    """quad j -> (strip jj4, psum col half hb, 4-row slot)."""
    return j % 4, j // 32, (j % 32) // 4


def _build_program_quad(use_b2, inv_b, inv_b2, inv_sout):
    """gelu(z) = z/2 + c z^2 + O(z^4) for |z|<<1, so each neuron's MLP
    collapses to out = weff.x + sum_k s_k (g_k.x)^2 with g_k = sqrt|l_k| v_k
    from eigh of the 32x32 quadratic form.  32 projections (not 64) on the
    PE, cheap squares (not gelu LUT) on ScalarE/DVE alternating whole units,
    and per-quad reduce matmuls (sgn on squares + weff on x) accumulate
    straight into the output PSUM."""
    ncores = int(os.environ.get("K_NCORES", NCORES))
    nc = bacc.Bacc("TRN2", target_bir_lowering=False, debug=False,
                   num_devices=ncores)

    # xp[32q+m, 256j+t] = x[t, 4j+q, m]  (fp16, quad-stacked)
    xp_d = nc.declare_dram_parameter("xp", [128, 64 * 256], _f16,
                                     isOutput=False)
    # per quad j (160 cols): g block [32q+m, 32q+k] (block-diag, fp8,
    # per-neuron pow2 scale) | sgn strip [32q+k, 4*slot+q] = +-comp_n
    w8_d = nc.declare_dram_parameter("w8", [128, 64 * 160], _f8,
                                     isOutput=False)
    # wf strip per quad (32 cols): [32q+m, 4*slot+q] = S_out * weff_n[m]
    wf_d = nc.declare_dram_parameter("wf", [128, 64 * 32], _f16,
                                     isOutput=False)
    if use_b2:
        b2_d = nc.declare_dram_parameter("b2bc", [128, 512], _f32,
                                         isOutput=False)
    # out[32jj4+4slot+q, 256hb+t] = y[t, 4j+q]
    out_d = nc.declare_dram_parameter("out", [128, 512], _f32, isOutput=True)

    with tile.TileContext(nc) as tc:
        with (
            tc.tile_pool(name="wpool", bufs=1) as wpool,
            tc.tile_pool(name="xpool", bufs=1) as xpool,
            tc.tile_pool(name="sqpool", bufs=4) as sqpool,
            tc.tile_pool(name="opool", bufs=1) as opool,
            tc.tile_pool(name="psab", bufs=3, space="PSUM") as psab,
            tc.tile_pool(name="psl2", bufs=1, space="PSUM") as psl2,
        ):
            # Input DMAs ride the two HWDGE queues: the 6 early-x chunks on
            # Scalar (done issuing before the squares ramp up), everything
            # else need-ordered on Sync.  Per-queue transfers are FIFO in
            # consumption order; two queues double the issue rate, which
            # bounds the startup ramp.
            xts = [None] * len(XQ_CHUNKS)
            w8ts = [None] * len(W8_CHUNKS)
            wfts = [None] * len(WF_CHUNKS)

            def _issue(queue, kind, i):
                eng = nc.scalar if queue == "A" else nc.sync
                if kind == "x":
                    s, L = XQ_CHUNKS[i]
                    t_ = xpool.tile([128, L * 512], _f16, name="xt",
                                    tag=f"xt{i}")
                    eng.dma_start(out=t_[:],
                                  in_=xp_d[:, s * 512:(s + L) * 512])
                    xts[i] = t_
                elif kind == "w8":
                    s, L = W8_CHUNKS[i]
                    t_ = wpool.tile([128, L * 320], _f8, name="w8t",
                                    tag=f"w8t{i}")
                    eng.dma_start(out=t_[:],
                                  in_=w8_d[:, s * 320:(s + L) * 320])
                    w8ts[i] = t_
                else:
                    s, L = WF_CHUNKS[i]
                    t_ = wpool.tile([128, L * 64], _f16, name="wft",
                                    tag=f"wft{i}")
                    eng.dma_start(out=t_[:],
                                  in_=wf_d[:, s * 64:(s + L) * 64])
                    wfts[i] = t_

            for queue, kind, i in _DMA_ORDER:
                _issue(queue, kind, i)
            b2sb = None
            if use_b2:
                b2sb = wpool.tile([128, 512], _f32, tag="b2sb")
                nc.sync.dma_start(out=b2sb[:], in_=b2_d[:])

            _emit_body_quad(nc, sqpool, opool, psab, psl2, out_d,
                            xts, w8ts, wfts, b2sb, inv_b, inv_b2, inv_sout)

    nc.finalize()
    return nc


def _emit_body_quad(nc, sqpool, opool, psab, psl2, out_d,
                    xts, w8ts, wfts, b2sb, inv_b, inv_b2, inv_sout):
    SQUARE = mybir.ActivationFunctionType.Square
    l2 = [psl2.tile([128, 512], _f32, name="l2", tag=f"l2{h}")
          for h in (0, 1)]

    zz = {}
    sq = {}

    def emit_l0(u):
        xk, xl = _XQMAP[u]
        wk, wl = _W8MAP[u]
        zz[u] = psab.tile([128, 512], _f32, name="zz", tag="zz")
        for qi in range(2):
            nc.tensor.matmul(
                zz[u][:, 256 * qi:256 * qi + 256],
                w8ts[wk][:, 320 * wl + 160 * qi:320 * wl + 160 * qi + 128],
                xts[xk][:, 512 * xl + 256 * qi:512 * xl + 256 * qi + 256],
                start=True, stop=True,
            )

    def emit_sq(u):
        sq[u] = sqpool.tile([128, 512], _f16, name="sq", tag="sq")
        if u % 2 == 0:
            nc.scalar.activation(sq[u][:], zz[u][:], SQUARE, scale=inv_b)
        else:
            nc.vector._custom_dve(
                _SQ_OP, out=sq[u][:], in0=zz[u][:],
                s0=inv_b2, s1=0.0, imm2=0.0)
        del zz[u]

    def emit_reduce(u0):
        # units u0, u0+1 -> quads 2u0..2u0+3 covering all 4 col strips
        for q in range(4):
            j = 2 * u0 + q
            u = u0 + q // 2
            qi = q % 2
            jj4, hb, _slot = _quad_slot(j)
            wk, wl = _W8MAP[u]
            fk, fl = _WFMAP[u]
            xk, xl = _XQMAP[u]
            # the first writer of each (strip, half) region uses start=True
            # (overwrite) in place of a zero-memset of the l2 banks
            first = j % 32 < 4
            nc.tensor.matmul(
                l2[hb][32 * jj4:32 * jj4 + 32, 0:256],
                w8ts[wk][:, 320 * wl + 160 * qi + 128:320 * wl + 160 * qi + 160],
                sq[u][:, 256 * qi:256 * qi + 256],
                start=first, stop=False,
                tile_position=(0, 32 * jj4),
                skip_group_check=True,
            )
            nc.tensor.matmul(
                l2[hb][32 * jj4:32 * jj4 + 32, 0:256],
                wfts[fk][:, 64 * fl + 32 * qi:64 * fl + 32 * qi + 32],
                xts[xk][:, 512 * xl + 256 * qi:512 * xl + 256 * qi + 256],
                start=False, stop=False,
                tile_position=(0, 32 * jj4),
                skip_group_check=True,
            )
        del sq[u0], sq[u0 + 1]

    def emit_evac(hb):
        o2 = opool.tile([128, 256], _f32, name="o2", tag=f"o2{hb}")
        nc.scalar.mul(o2[:], l2[hb][:, 0:256], inv_sout)
        if b2sb is not None:
            nc.vector.tensor_add(o2[:], o2[:], b2sb[:, 256 * hb:256 * hb + 256])
        nc.sync.dma_start(out=out_d[:, 256 * hb:256 * hb + 256], in_=o2[:])

    for t in range(NUNIT + 3):
        if t < NUNIT:
            emit_l0(t)
        if 0 <= t - 1 < NUNIT:
            emit_sq(t - 1)
        if t >= 4 and (t - 4) % 2 == 0 and t - 4 < NUNIT:
            emit_reduce(t - 4)
        if t == 21:
            # quads 0..31 (units 0-15) all reduced by t=20 -> stream out
            # the first output half while the back half still computes.
            emit_evac(0)
    emit_evac(1)


def _lin_ok(x, W0, b0, W1, b1):
    """gelu(z1) ~= z1/2 only holds when |z1| << 1; estimate max|z1| on a
    small batch sample (tanh-gelu approx is fine for a magnitude check)."""
    if bool(np.any(b1)):
        return False
    xs = x[:8].astype(np.float32)
    z0 = np.einsum('bdm,dmh->bdh', xs, W0.astype(np.float32))
    if bool(np.any(b0)):
        z0 = z0 + b0[None].astype(np.float32)
    h0 = 0.5 * z0 * (1.0 + np.tanh(0.7978845608 * (z0 + 0.044715 * z0**3)))
    z1 = np.einsum('bdh,dho->bdo', h0, W1.astype(np.float32))
    return float(np.abs(z1).max()) < 0.005


def _quad_ok(x, W0, b0, W1, b1):
    """The quadratic-gelu path additionally needs |z0| << 1."""
    if bool(np.any(b0)) or bool(np.any(b1)):
        return False
    if not _lin_ok(x, W0, b0, W1, b1):
        return False
    xs = x[:8].astype(np.float32)
    z0 = np.einsum('bdm,dmh->bdh', xs, W0.astype(np.float32))
    return float(np.abs(z0).max()) < 0.15


def _prep_quad_host(x, W0, W1, W2):
    """Global (all-neuron) eigendecomposition of the per-neuron quadratic
    form + pow2 scale selection."""
    import ml_dtypes
    f8 = ml_dtypes.float8_e4m3fn
    GC = 0.3989422804014327
    W0d = W0.astype(np.float64)
    veff = 0.5 * np.einsum('dho,do->dh', W1.astype(np.float64),
                           W2[:, :, 0].astype(np.float64))
    weff = 0.5 * np.einsum('dmh,dh->dm', W0d, veff)
    Q = GC * np.einsum('dmh,dh,dnh->dmn', W0d, veff, W0d)
    lam, V = np.linalg.eigh(Q)
    g = np.sqrt(np.abs(lam))[:, None, :] * V        # [D, m, 32]
    sgn = np.sign(lam)
    mx = np.maximum(np.abs(g).max(axis=(1, 2)), 1e-30)
    gs = 2.0 ** np.clip(np.round(np.log2(0.25 / mx)), -40, 40)
    gq = (g * gs[:, None, None]).astype(f8)
    zs = np.einsum('bdm,dmk->bdk',
                   x[:16].astype(np.float16).astype(np.float64),
                   gq.astype(np.float64))
    zmax = float(np.abs(zs).max()) * 1.5
    Bq = 2.0 ** np.ceil(np.log2(max(zmax, 1e-6) / 16.0))
    gs_med = float(np.median(gs))
    S_out = 2.0 ** np.round(np.log2((gs_med / Bq) ** 2))
    comp = S_out * Bq * Bq / gs ** 2                # pow2 per neuron
    assert comp.max() <= 256.0 and comp.min() >= 2.0 ** -9, (
        "comp outside fp8 range", comp.min(), comp.max())
    sgnq = (sgn * comp[:, None]).astype(f8)
    wfq = (weff * S_out).astype(np.float16)
    assert np.abs(wfq).max() < 60000.0, "wf overflow"
    return gq, sgnq, wfq, Bq, S_out


def _pack_core_quad(x, gq, sgnq, wfq, b2, c, use_b2):
    import ml_dtypes
    sl = slice(ND * c, ND * (c + 1))
    xc = x[:, sl, :]                                   # [B, 256, 32]
    xp = xc.transpose(1, 2, 0).reshape(64, 128, B)
    xp = np.ascontiguousarray(
        xp.transpose(1, 0, 2)).reshape(128, 64 * B).astype(np.float16)
    gqc, sgc, wfc = gq[sl], sgnq[sl], wfq[sl]
    w8 = np.zeros((128, 64 * 160), ml_dtypes.float8_e4m3fn)
    wf = np.zeros((128, 64 * 32), np.float16)
    for j in range(64):
        jj4, hb, slot = _quad_slot(j)
        for q in range(4):
            n = 4 * j + q
            w8[32 * q:32 * q + 32,
               160 * j + 32 * q:160 * j + 32 * q + 32] = gqc[n]
            w8[32 * q:32 * q + 32, 160 * j + 128 + 4 * slot + q] = sgc[n]
            wf[32 * q:32 * q + 32, 32 * j + 4 * slot + q] = wfc[n]
    m = {"xp": xp, "w8": w8, "wf": wf}
    if use_b2:
        b2bc = np.zeros((128, 512), np.float32)
        b2row = b2[sl, 0].astype(np.float32)
        for j in range(64):
            jj4, hb, slot = _quad_slot(j)
            for q in range(4):
                b2bc[32 * jj4 + 4 * slot + q,
                     256 * hb:256 * hb + 256] = b2row[4 * j + q]
        m["b2bc"] = b2bc
    return m


def _unstitch_quad(o):
    """o [128,512]: out[32jj4+4slot+q, 256hb+t] = y[t, 128hb+16slot+4jj4+q]."""
    o5 = o.reshape(4, 8, 4, 2, 256)                    # [jj4, slot, q, hb, t]
    return np.ascontiguousarray(
        o5.transpose(4, 3, 1, 0, 2)).reshape(256, 256)


def _get_program_quad(use_b2, Bq, S_out):
    key = ("quad", use_b2, Bq, S_out,
           os.environ.get("K_NCORES"), os.environ.get("K_NREP"))
    if key not in _PROGRAM_CACHE:
        _PROGRAM_CACHE[key] = _build_program_quad(
            use_b2, 1.0 / Bq, 1.0 / (Bq * Bq), 1.0 / S_out)
    return _PROGRAM_CACHE[key]


def _make_plan(x, W0, b0, W1, b1, W2, b2):
    """Shared by kernel() and test.py: returns (nc, in_maps, post)."""
    ncores = int(os.environ.get("K_NCORES", NCORES))
    use_b0 = bool(np.any(b0))
    use_b1 = bool(np.any(b1))
    use_b2 = bool(np.any(b2))
    if _quad_ok(x, W0, b0, W1, b1):
        gq, sgnq, wfq, Bq, S_out = _prep_quad_host(x, W0, W1, W2)
        nc = _get_program_quad(use_b2, Bq, S_out)
        in_maps = [_pack_core_quad(x, gq, sgnq, wfq, b2, c, use_b2)
                   for c in range(ncores)]
        post = _unstitch_quad
    else:
        use_lin = _lin_ok(x, W0, b0, W1, b1)
        nc = _get_program(use_b0, use_b1, use_b2, use_lin)
        in_maps = [
            _prep_core(x, W0, b0, W1, b1, W2, b2, c, use_b0, use_b1, use_b2,
                       use_lin)
            for c in range(ncores)
        ]
        post = _unstitch
    return nc, in_maps, post


def _prep_core(x, W0, b0, W1, b1, W2, b2, c, use_b0, use_b1, use_b2=False,
               use_lin=False):
    sl = slice(ND * c, ND * (c + 1))
    # xp[32q+m, 256j+t] = x[t, 4j+q, m]
    xc = x[:, sl, :]                                   # [B, 256, 32]
    xp = xc.transpose(1, 2, 0).reshape(64, 128, B)     # [j, 32q+m, t]
    xp = np.ascontiguousarray(
        xp.transpose(1, 0, 2)).reshape(128, 64 * B).astype(np.float16)
    # packed per-unit weights
    ucols = 384 if use_lin else 640
    wall = np.zeros((128, NUNIT * ucols), np.float16)
    W0c = W0[sl].astype(np.float16)                    # [256, 32, 64]
    if use_lin:
        # veff[d] = S_V * (W1[d] @ W2[d]) / 2  -- folds L1+gelu1+L2
        vc = (S_V * 0.5 * np.einsum(
            'dho,do->dh', W1[sl].astype(np.float64),
            W2[sl, :, 0].astype(np.float64))).astype(np.float16)  # [256, 64]
    else:
        W1c = W1[sl].astype(np.float16)                # [256, 64, 64]
        w2c = W2[sl, :, 0].astype(np.float16)          # [256, 64]
    for u in range(NUNIT):
        base = ucols * u
        for jj in range(2):                            # stack j = 2u+jj
            j = 2 * u + jj
            for a in range(2):
                for b in range(2):
                    r = 64 * a + 32 * b
                    cc = base + 128 * jj + 64 * b
                    wall[r:r + 32, cc:cc + 64] = W0c[4 * j + 2 * a + b]
        for c in range(4):
            p = 4 * u + c
            _, _, m_ = _l2slot(p)
            if use_lin:
                for e in range(2):
                    wall[64 * e:64 * e + 64,
                         base + 256 + 32 * c + 2 * m_ + e] = vc[2 * p + e]
            else:
                for b in range(2):
                    wall[64 * b:64 * b + 64,
                         base + 256 + 64 * c:base + 320 + 64 * c] = (
                        W1c[2 * p + b])
                for e in range(2):
                    wall[64 * e:64 * e + 64,
                         base + 512 + 32 * c + 2 * m_ + e] = w2c[2 * p + e]
    m = {"xp": xp, "wall": wall}
    if use_b2:
        # b2bc[32j+2m+e, 256hb+t] = b2[16m+8hb+2j+e]
        b2bc = np.zeros((128, 512), np.float32)
        b2row = b2[sl, 0].astype(np.float32)
        for p in range(NPAIR):
            j, hb, m_ = _l2slot(p)
            for e in range(2):
                b2bc[32 * j + 2 * m_ + e, 256 * hb:256 * hb + 256] = (
                    b2row[2 * p + e])
        m["b2bc"] = b2bc
    if use_b0:
        b0p = b0[sl].reshape(NPAIR, 2, H).transpose(1, 2, 0)
        m["b0p"] = np.ascontiguousarray(b0p).reshape(128, NPAIR).astype(np.float32)
    if use_b1:
        b1p = b1[sl].reshape(NPAIR, 2, H).transpose(1, 2, 0)
        m["b1p"] = np.ascontiguousarray(b1p).reshape(128, NPAIR).astype(np.float32)
    return m


def _unstitch(o):
    """o [128,512]: out[32j+2m+e, 256hb+t] = y[t, 16m+8hb+2j+e]."""
    o5 = o.reshape(4, 16, 2, 2, 256)                   # [j, m, e, hb, t]
    return np.ascontiguousarray(
        o5.transpose(4, 1, 3, 0, 2)).reshape(256, 256)  # [t, m,hb,j,e]


def kernel(pre_activation_history, W0, b0, W1, b1, W2, b2):
    x = np.asarray(pre_activation_history, np.float32)
    W0 = np.asarray(W0, np.float32)
    b0 = np.asarray(b0, np.float32)
    W1 = np.asarray(W1, np.float32)
    b1 = np.asarray(b1, np.float32)
    W2 = np.asarray(W2, np.float32)
    b2 = np.asarray(b2, np.float32)

    nc, in_maps, post = _make_plan(x, W0, b0, W1, b1, W2, b2)
    ncores = int(os.environ.get("K_NCORES", NCORES))
    res = run_bass_kernel_spmd(nc, in_maps, list(range(ncores)))
    y = np.zeros((B, D), np.float32)
    for c in range(ncores):
        y[:, ND * c:ND * (c + 1)] = post(res.results[c]["out"])
    return y



# revision 30
# speedup vs baseline: 1.0176x; 1.0176x over previous
"""Trainium2 Bass kernel for per-neuron MLPs (dense_mlp).

reference: out[b,d] = W2[d]^T.gelu(W1[d]^T.gelu(W0[d]^T.x[b,d,:]+b0)+b1)+b2
Shapes: x [256,2048,32], W0 [2048,32,64], W1 [2048,64,64], W2 [2048,64,1].

Sharding: D split across 8 cores (256 neurons each, fully independent).

Quadratic fast path (gated by _quad_ok): for this problem both hidden
pre-activations are tiny (|z0| < 0.1, |z1| < 5e-3), so
  gelu(z1) ~= z1/2          (collapses L1+gelu1+L2 into veff = W1@W2/2)
  gelu(z0) ~= z0/2 + c z0^2 (c = 1/sqrt(2pi); quartic term ~1e-5 rel)
and each neuron's whole MLP becomes
  out_d(x) = weff_d.x + sum_k s_k (g_k.x)^2
where Q_d = c W0 diag(veff) W0^T (32x32) = V diag(lam) V^T (host eigh),
g_k = sqrt|lam_k| v_k (fp8, per-neuron pow2 scale), s_k = sign(lam_k)
(carried as +-pow2 compensation in fp8), weff = W0.veff/2 (fp16).
This halves PE projections (32/neuron, not 64) and replaces the gelu LUT
stage with one cheap square per PSUM bank.  End-to-end rel err ~1.4e-3
vs the 2e-2 gate; the older lin/full pipelines remain as fallbacks.

Per-core dataflow (unit = 8 neurons = 2 quads, software-pipelined:
step t emits proj(t) | square(t-1) | reduce(t-4, pairs of units)):
  DMA: 17 chunks on BOTH HWDGE queues in consumption order - early-x on
      Scalar's queue, weights + late-x need-ordered on Sync's.  5.75MB
      total per core (x fp16 4MB, g/sgn fp8 1.25MB, weff fp16 0.5MB).
  proj: per quad one full-array matmul: block-diag g lhsT [128,128] fp8
      (rows 32q+m, cols 32q+k) x x-quad-stack [128,256] fp16 -> one PSUM
      bank zz [128,512] per unit (two quads side by side).
  square: whole units alternate ScalarE (Square LUT, scale 1/B) and DVE
      (custom C0*u^2 op) -> sq [128,512] fp16; one op per unit because
      the ~400ns fixed PSUM-access overhead dominates op size.
  reduce: per quad TWO 32-col-strip matmuls at tile_position (0,32(j%4))
      accumulate into l2[j//32] [128,512]: sgn strip (+-comp pow2) x sq
      gives the quadratic term; weff strip x the same x tile gives the
      linear term.  First writer per strip uses start=True (no memset).
      Batches of 4 quads keep all 4 column strips concurrently busy.
  evac: half 0 (quads 0-31) streams out at t=21 overlapping compute;
      half 1 after the loop.  o2 = l2 * (1/S_out) on ScalarE (+b2).
  Host re-stitches out[32(j%4)+4slot+q, 256hb+t] -> y[B, ND].
"""

import os
import sys

for _p in ("/opt/trn_rl_repo",):
    if _p not in sys.path:
        sys.path.insert(0, _p)

import numpy as np

import concourse.dve_ops as _dvo
from concourse import bacc, mybir, tile
from concourse import bass_utils as _bu
from concourse.bass_utils import run_bass_kernel_spmd


from concourse.dve_ops import DveOp, DveOpSpec, has_src1, lower as _dve_lower
from concourse.dve_spec import Spec, Src0, C0, C1, C2, One, sq

B = 256
D = 2048
M = 32
H = 64
NCORES = 8
ND = D // NCORES          # neurons per core = 256
NPAIR = ND // 2           # 128
NUNIT = ND // 8           # 32 units of 8 neurons (4 pairs)
GELU_C = 0.3989422804014327  # 1/sqrt(2*pi)
S_H1 = float(2 ** 14)     # fp16 scale for h1 (values ~1e-4 -> ~1.6)
S_V = float(2 ** 9)       # fp16 scale for veff = W1@W2/2 (values ~3e-5)

_f32 = mybir.dt.float32
_f16 = mybir.dt.float16


def _zc(c):
    """z0/h0 column of pair-in-unit c; concurrent row groups (c%2) get
    different PSUM banks."""
    return 512 * (c % 2) + 256 * (c // 2)


def _l2slot(p):
    """pair p -> (strip j, col half hb, partition slot m) in l2ps."""
    return p % 4, (p // 4) % 2, p // 8


_CH = [(0, 1), (1, 1), (2, 2), (4, 4), (8, 8), (16, 8), (24, 8)]
X_CHUNKS = list(_CH)
W_CHUNKS = list(_CH)


def _chunk_map(chunks):
    m = {}
    for k, (s, L) in enumerate(chunks):
        for u in range(s, s + L):
            m[u] = (k, u - s)
    return m


_XMAP = _chunk_map(X_CHUNKS)
_WMAP = _chunk_map(W_CHUNKS)


def _register_gelu_op():
    """out = u*(C1 + u*C0*(1 + u^2*C2)); with C0=S*c, C1=S/2, C2=-1/6 this is
    S*gelu(u) up to O(u^6) of the exact erf-gelu Taylor series."""
    name = "GELU_SCALED_ANT"
    for op in _dvo.OPS:
        if op.name == name:
            return op
    u = Src0
    body = u * (C1 + u * C0 * (One + sq(u) * C2))
    spec = Spec(
        body=body,
        reference=lambda in0, s0, s1, imm2: in0
        * (s1 + in0 * s0 * (1.0 + (in0 * in0) * imm2)),
    )
    shas = {}
    op = DveOp(name, spec, subdim=False, uops_sha=shas)
    _dvo.OPS.append(op)
    _dvo.CUSTOM_DVE_SPECS[name] = spec
    _dvo._SUB_OPCODE_FOR_NAME[name] = _dvo._CUSTOM_DVE_ROW_BASE + len(_dvo.OPS) - 1
    for ver in ("v3", "v4"):
        tmp = DveOpSpec(
            name=name,
            opcode=_dvo.get_dve_sub_opcode(name),
            uops=_dve_lower(spec, ver=ver),
            rd1_en=has_src1(spec),
        )
        shas[ver] = tmp.sha(ver)
    return op


_GELU_OP = _register_gelu_op()


def _register_sq_op():
    """out = C0 * Src0^2 — scaled square for the quadratic-gelu path."""
    name = "SQSCALE_ANT"
    for op in _dvo.OPS:
        if op.name == name:
            return op
    body = sq(Src0) * C0
    spec = Spec(
        body=body,
        reference=lambda in0, s0, s1, imm2: in0 * in0 * s0,
    )
    shas = {}
    op = DveOp(name, spec, subdim=False, uops_sha=shas)
    _dvo.OPS.append(op)
    _dvo.CUSTOM_DVE_SPECS[name] = spec
    _dvo._SUB_OPCODE_FOR_NAME[name] = _dvo._CUSTOM_DVE_ROW_BASE + len(_dvo.OPS) - 1
    for ver in ("v3", "v4"):
        tmp = DveOpSpec(
            name=name,
            opcode=_dvo.get_dve_sub_opcode(name),
            uops=_dve_lower(spec, ver=ver),
            rd1_en=has_src1(spec),
        )
        shas[ver] = tmp.sha(ver)
    return op


_SQ_OP = _register_sq_op()

_PROGRAM_CACHE = {}


def _build_program(use_b0, use_b1, use_b2, use_lin=False):
    ncores = int(os.environ.get("K_NCORES", NCORES))
    nrep = int(os.environ.get("K_NREP", 1))
    nc = bacc.Bacc("TRN2", target_bir_lowering=False, debug=False,
                   num_devices=ncores)

    ucols = 384 if use_lin else 640
    # x pair-stacks: xp[32q+m, 256j+t] = x[t, 4j+q, m]
    xp_d = nc.declare_dram_parameter("xp", [128, 64 * 256], _f16,
                                     isOutput=False)
    # all weights packed per unit.
    # full path (640 cols/unit: w0 256 | w1 256 | w2 128):
    #   w0 block: [64a+32b+m, 128*(j-2u)+64b+h] = W0[4j+2a+b][m,h]
    #   w1 block: [64b+h, 64c+o] = W1[2(4u+c)+b][h,o]
    #   w2 block: zero-padded blockdiag [64e+h, 32c+2m+e] = W2[2(4u+c)+e][h]
    # linearized path (384 cols/unit: w0 256 | veff 128), where
    #   veff[d] = S_V * (W1[d] @ W2[d]) / 2 replaces w1/w2 blocks.
    wall_d = nc.declare_dram_parameter("wall", [128, NUNIT * ucols], _f16,
                                       isOutput=False)
    if use_b2:
        b2_d = nc.declare_dram_parameter("b2bc", [128, 512], _f32,
                                         isOutput=False)
    if use_b0:
        # b0p[64b+h, p] = b0[2p+b][h]
        b0_d = nc.declare_dram_parameter("b0p", [128, NPAIR], _f32,
                                         isOutput=False)
    if use_b1:
        b1_d = nc.declare_dram_parameter("b1p", [128, NPAIR], _f32,
                                         isOutput=False)
    # out[32j+2m+e, 256hb+t] = y[t, 16m+8hb+2j+e]
    out_d = nc.declare_dram_parameter("out", [128, 512], _f32, isOutput=True)

    GELU = mybir.ActivationFunctionType.Gelu

    with tile.TileContext(nc) as tc:
        with (
            tc.tile_pool(name="wpool", bufs=1) as wpool,
            tc.tile_pool(name="xpool", bufs=1) as xpool,
            tc.tile_pool(name="h0pool", bufs=3) as h0pool,
            tc.tile_pool(name="h1pool", bufs=3) as h1pool,
            tc.tile_pool(name="opool", bufs=1) as opool,
            tc.tile_pool(name="psab", bufs=3, space="PSUM") as psab,
            tc.tile_pool(name="ps2", bufs=1, space="PSUM") as ps2,
        ):
            # Geometric unit-granular chunks; x chunks issue on the Sync
            # HWDGE queue, weight chunks on the Scalar HWDGE queue so the
            # two streams transfer concurrently and each queue only pays
            # ~650ns issue cost per chunk (7 chunks/queue, not 33 on one).
            xts = []
            wts = []
            deferred = []

            for i in range(max(len(X_CHUNKS), len(W_CHUNKS))):
                if i < len(X_CHUNKS):
                    s, L = X_CHUNKS[i]
                    xt = xpool.tile([128, L * 512], _f16, name="xt",
                                    tag=f"xt{i}")
                    nc.sync.dma_start(out=xt[:], in_=xp_d[:, s * 512:(s + L) * 512])
                    xts.append(xt)
                if i < len(W_CHUNKS):
                    s, L = W_CHUNKS[i]
                    wt = wpool.tile([128, L * ucols], _f16, name="wt",
                                    tag=f"wt{i}")
                    nc.scalar.dma_start(out=wt[:],
                                        in_=wall_d[:, s * ucols:(s + L) * ucols])
                    wts.append(wt)
            b0sb = b1sb = b2sb = None
            if use_b2:
                b2sb = wpool.tile([128, 512], _f32, tag="b2sb")
                nc.sync.dma_start(out=b2sb[:], in_=b2_d[:])
            if use_b0:
                b0sb = wpool.tile([128, NPAIR], _f32, tag="b0sb")
                nc.sync.dma_start(out=b0sb[:], in_=b0_d[:])
            if use_b1:
                b1sb = wpool.tile([128, NPAIR], _f32, tag="b1sb")
                nc.sync.dma_start(out=b1sb[:], in_=b1_d[:])

            for _rep in range(nrep):
                if use_lin:
                    _emit_body_lin(nc, h0pool, opool, psab, ps2,
                                   out_d, xts, wts, b0sb, b2sb, GELU,
                                   deferred)
                else:
                    _emit_body(nc, h0pool, h1pool, opool, psab, ps2,
                               out_d, xts, wts, b0sb, b1sb, b2sb, GELU)

    nc.finalize()
    return nc


def _emit_body_lin(nc, h0pool, opool, psab, ps2,
                   out_d, xts, wts, b0sb, b2sb, GELU, deferred=()):
    """gelu(z1) ~= z1/2 for |z1| << 1, so L1+gelu1+L2 collapse into one
    per-neuron vector veff = W1 @ W2 / 2 applied to h0 with the same
    zero-padded block-diag accumulate as L2."""
    l2ps = ps2.tile([128, 512], _f32, tag="l2")
    nc.vector.memset(l2ps[:], 0.0)

    z0 = {}
    h0 = {}

    def emit_l0(u):
        xk, xl = _XMAP[u]
        wk, wl = _WMAP[u]
        xt = xts[xk]
        wt = wts[wk]
        z0[u] = psab.tile([128, 1024], _f32, name="z0", tag="zz")
        for c in range(4):
            a = c % 2
            wcol = 384 * wl + 128 * (c // 2)
            xcol = 512 * xl + 256 * (c // 2)
            nc.tensor.matmul(
                z0[u][:, _zc(c):_zc(c) + 256],
                wt[64 * a:64 * a + 64, wcol:wcol + 128],
                xt[64 * a:64 * a + 64, xcol:xcol + 256],
                start=True, stop=True,
                tile_position=(64 * a, 0),
            )

    def emit_gelu0(u):
        # split across engines: ScalarE bank A (cols 0-511), DVE custom
        # poly bank B (cols 512-1023) -- parallel PSUM access, balanced
        # ~720ns vs ~680ns.
        h0[u] = h0pool.tile([128, 1024], _f16, name="h0", tag="h0")
        if b0sb is not None:
            for c in range(4):            # correct fallback: all-ScalarE
                p = 4 * u + c
                nc.scalar.activation(
                    h0[u][:, _zc(c):_zc(c) + 256],
                    z0[u][:, _zc(c):_zc(c) + 256],
                    GELU, bias=b0sb[:, p:p + 1], scale=1.0)
        else:
            nc.scalar.activation(h0[u][:, 0:512], z0[u][:, 0:512], GELU)
            nc.vector._custom_dve(
                _GELU_OP, out=h0[u][:, 512:1024], in0=z0[u][:, 512:1024],
                s0=GELU_C, s1=0.5, imm2=-1.0 / 6.0)
        del z0[u]

    def emit_l12(u):
        wk, wl = _WMAP[u]
        for c in range(4):
            p = 4 * u + c
            j, hb, m_ = _l2slot(p)
            wcol = 384 * wl + 256 + 32 * c
            nc.tensor.matmul(
                l2ps[32 * j:32 * j + 32, 256 * hb:256 * hb + 256],
                wts[wk][:, wcol:wcol + 32],
                h0[u][:, _zc(c):_zc(c) + 256],
                start=False, stop=False,
                tile_position=(0, 32 * j),
                skip_group_check=True,
            )
        del h0[u]

    for t in range(NUNIT + 2):
        if t < NUNIT:
            emit_l0(t)
        if 0 <= t - 1 < NUNIT:
            emit_gelu0(t - 1)
            for g, tile_, dram, lo, hi in deferred:
                if g == t:
                    # tiny GpSimd write orders the DMA (WAW) behind
                    # pipeline progress, keeping its transfer out of the
                    # startup ramp's bandwidth window
                    nc.gpsimd.tensor_copy(tile_[0:1, 0:2],
                                          h0[t - 1][0:1, 0:2])
                    nc.sync.dma_start(out=tile_[:], in_=dram[:, lo:hi])
        if 0 <= t - 2 < NUNIT:
            emit_l12(t - 2)

    o2 = opool.tile([128, 512], _f32, tag="o2")
    nc.scalar.mul(o2[:], l2ps[:], 1.0 / S_V)
    if b2sb is not None:
        nc.vector.tensor_add(o2[:], o2[:], b2sb[:])
    nc.sync.dma_start(out=out_d[:], in_=o2[:])


def _emit_body(nc, h0pool, h1pool, opool, psab, ps2,
               out_d, xts, wts, b0sb, b1sb, b2sb, GELU):
    l2ps = ps2.tile([128, 512], _f32, tag="l2")
    # Data is zeroed up front so every L2 matmul can use start=False:
    # first-writer overwrite and accumulate both produce 0 + v.
    nc.vector.memset(l2ps[:], 0.0)

    z0 = {}
    h0 = {}
    h1 = {}

    def emit_l0(u):
        xk, xl = _XMAP[u]
        wk, wl = _WMAP[u]
        xt = xts[xk]
        wt = wts[wk]
        z0[u] = psab.tile([128, 1024], _f32, name="z0", tag="zz")
        for c in range(4):
            a = c % 2
            wcol = 640 * wl + 128 * (c // 2)
            xcol = 512 * xl + 256 * (c // 2)
            nc.tensor.matmul(
                z0[u][:, _zc(c):_zc(c) + 256],
                wt[64 * a:64 * a + 64, wcol:wcol + 128],
                xt[64 * a:64 * a + 64, xcol:xcol + 256],
                start=True, stop=True,
                tile_position=(64 * a, 0),
            )

    def emit_gelu0(u):
        h0[u] = h0pool.tile([128, 1024], _f16, name="h0", tag="h0")
        if b0sb is not None:
            for c in range(4):
                p = 4 * u + c
                nc.scalar.activation(
                    h0[u][:, _zc(c):_zc(c) + 256],
                    z0[u][:, _zc(c):_zc(c) + 256],
                    GELU, bias=b0sb[:, p:p + 1], scale=1.0)
        else:
            nc.scalar.activation(h0[u][:], z0[u][:], GELU)
        del z0[u]

    def emit_l1_gelu1(u):
        z1 = psab.tile([128, 1024], _f32, name="z1", tag="zz")
        for c in range(4):
            p = 4 * u + c
            for b in range(2):
                rp = 64 * b
                wk, wl = _WMAP[u]
                nc.tensor.matmul(
                    z1[rp:rp + 64, 256 * c:256 * c + 256],
                    wts[wk][rp:rp + 64,
                            640 * wl + 256 + 64 * c:640 * wl + 320 + 64 * c],
                    h0[u][rp:rp + 64, _zc(c):_zc(c) + 256],
                    start=True, stop=True,
                    tile_position=(rp, rp),
                )
        gelu_in = z1
        if b1sb is not None:
            tmp = h0pool.tile([128, 1024], _f32, name="b1tmp", tag="b1tmp")
            for c in range(4):
                p = 4 * u + c
                nc.vector.tensor_scalar_add(
                    tmp[:, 256 * c:256 * c + 256],
                    z1[:, 256 * c:256 * c + 256],
                    b1sb[:, p:p + 1])
            gelu_in = tmp
        h1[u] = h1pool.tile([128, 1024], _f16, name="h1", tag="h1")
        nc.vector._custom_dve(
            _GELU_OP, out=h1[u][:], in0=gelu_in[:],
            s0=S_H1 * GELU_C, s1=S_H1 * 0.5, imm2=-1.0 / 6.0)
        del h0[u]

    def emit_l2(u):
        for c in range(4):
            p = 4 * u + c
            j, hb, m_ = _l2slot(p)
            ht = h1[u]
            wk, wl = _WMAP[u]
            wcol = 640 * wl + 512 + 32 * c
            nc.tensor.matmul(
                l2ps[32 * j:32 * j + 32, 256 * hb:256 * hb + 256],
                wts[wk][:, wcol:wcol + 32],
                ht[:, 256 * c:256 * c + 256],
                start=False, stop=False,
                tile_position=(0, 32 * j),
                skip_group_check=True,
            )
        del h1[u]

    for t in range(NUNIT + 3):
        if t < NUNIT:
            emit_l0(t)
        if 0 <= t - 1 < NUNIT:
            emit_gelu0(t - 1)
            emit_l1_gelu1(t - 1)
        if 0 <= t - 3 < NUNIT:
            emit_l2(t - 3)

    # ---- evac + store ----
    o2 = opool.tile([128, 512], _f32, tag="o2")
    nc.scalar.mul(o2[:], l2ps[:], 1.0 / S_H1)
    if b2sb is not None:
        nc.vector.tensor_add(o2[:], o2[:], b2sb[:])
    nc.sync.dma_start(out=out_d[:], in_=o2[:])


XQ_CHUNKS = [(0, 1), (1, 2), (3, 3), (6, 4), (10, 4), (14, 4), (18, 5),
             (23, 5), (28, 4)]
W8_CHUNKS = [(0, 1), (1, 4), (5, 8), (13, 9), (22, 10)]
WF_CHUNKS = [(0, 8), (8, 12), (20, 12)]
_XQMAP = _chunk_map(XQ_CHUNKS)
_W8MAP = _chunk_map(W8_CHUNKS)
_WFMAP = _chunk_map(WF_CHUNKS)
# x chunks 0-5 issue on the Scalar HWDGE queue; the rest on Sync in
# first-need order (wf chunk i is first needed ~4 units past its start)
_DMA_ORDER = [("w8", 0), ("x", 0), ("w8", 1), ("x", 1), ("wf", 0),
              ("x", 2), ("w8", 2), ("x", 3), ("wf", 1), ("x", 4),
              ("w8", 3), ("x", 5), ("x", 6), ("w8", 4), ("x", 7),
              ("wf", 2), ("x", 8)]
_f8 = mybir.dt.float8e4


def _quad_slot(j):
    """quad j -> (strip jj4, psum col half hb, 4-row slot)."""
    return j % 4, j // 32, (j % 32) // 4


def _build_program_quad(use_b2, inv_b, inv_b2, inv_sout):
    """gelu(z) = z/2 + c z^2 + O(z^4) for |z|<<1, so each neuron's MLP
    collapses to out = weff.x + sum_k s_k (g_k.x)^2 with g_k = sqrt|l_k| v_k
    from eigh of the 32x32 quadratic form.  32 projections (not 64) on the
    PE, cheap squares (not gelu LUT) on ScalarE/DVE alternating whole units,
    and per-quad reduce matmuls (sgn on squares + weff on x) accumulate
    straight into the output PSUM."""
    ncores = int(os.environ.get("K_NCORES", NCORES))
    nc = bacc.Bacc("TRN2", target_bir_lowering=False, debug=False,
                   num_devices=ncores)

    # xp[32q+m, 256j+t] = x[t, 4j+q, m]  (fp16, quad-stacked)
    xp_d = nc.declare_dram_parameter("xp", [128, 64 * 256], _f16,
                                     isOutput=False)
    # per quad j (160 cols): g block [32q+m, 32q+k] (block-diag, fp8,
    # per-neuron pow2 scale) | sgn strip [32q+k, 4*slot+q] = +-comp_n
    w8_d = nc.declare_dram_parameter("w8", [128, 64 * 160], _f8,
                                     isOutput=False)
    # wf strip per quad (32 cols): [32q+m, 4*slot+q] = S_out * weff_n[m]
    wf_d = nc.declare_dram_parameter("wf", [128, 64 * 32], _f16,
                                     isOutput=False)
    if use_b2:
        b2_d = nc.declare_dram_parameter("b2bc", [128, 512], _f32,
                                         isOutput=False)
    # out[32jj4+4slot+q, 256hb+t] = y[t, 4j+q]
    out_d = nc.declare_dram_parameter("out", [128, 512], _f32, isOutput=True)

    with tile.TileContext(nc) as tc:
        with (
            tc.tile_pool(name="wpool", bufs=1) as wpool,
            tc.tile_pool(name="xpool", bufs=1) as xpool,
            tc.tile_pool(name="sqpool", bufs=4) as sqpool,
            tc.tile_pool(name="opool", bufs=1) as opool,
            tc.tile_pool(name="psab", bufs=3, space="PSUM") as psab,
            tc.tile_pool(name="psl2", bufs=1, space="PSUM") as psl2,
        ):
            # Input DMAs ride the two HWDGE queues: the 6 early-x chunks on
            # Scalar (done issuing before the squares ramp up), everything
            # else need-ordered on Sync.  Per-queue transfers are FIFO in
            # consumption order; two queues double the issue rate, which
            # bounds the startup ramp.
            xts = [None] * len(XQ_CHUNKS)
            w8ts = [None] * len(W8_CHUNKS)
            wfts = [None] * len(WF_CHUNKS)

            def _issue(kind, i):
                if kind == "x":
                    s, L = XQ_CHUNKS[i]
                    t_ = xpool.tile([128, L * 512], _f16, name="xt",
                                    tag=f"xt{i}")
                    eng = nc.scalar if i < 6 else nc.sync
                    eng.dma_start(out=t_[:],
                                  in_=xp_d[:, s * 512:(s + L) * 512])
                    xts[i] = t_
                elif kind == "w8":
                    s, L = W8_CHUNKS[i]
                    t_ = wpool.tile([128, L * 320], _f8, name="w8t",
                                    tag=f"w8t{i}")
                    nc.sync.dma_start(out=t_[:],
                                      in_=w8_d[:, s * 320:(s + L) * 320])
                    w8ts[i] = t_
                else:
                    s, L = WF_CHUNKS[i]
                    t_ = wpool.tile([128, L * 64], _f16, name="wft",
                                    tag=f"wft{i}")
                    nc.sync.dma_start(out=t_[:],
                                      in_=wf_d[:, s * 64:(s + L) * 64])
                    wfts[i] = t_

            for kind, i in _DMA_ORDER:
                _issue(kind, i)
            b2sb = None
            if use_b2:
                b2sb = wpool.tile([128, 512], _f32, tag="b2sb")
                nc.sync.dma_start(out=b2sb[:], in_=b2_d[:])

            _emit_body_quad(nc, sqpool, opool, psab, psl2, out_d,
                            xts, w8ts, wfts, b2sb, inv_b, inv_b2, inv_sout)

    nc.finalize()
    return nc


def _emit_body_quad(nc, sqpool, opool, psab, psl2, out_d,
                    xts, w8ts, wfts, b2sb, inv_b, inv_b2, inv_sout):
    SQUARE = mybir.ActivationFunctionType.Square
    l2 = [psl2.tile([128, 512], _f32, name="l2", tag=f"l2{h}")
          for h in (0, 1)]

    zz = {}
    sq = {}

    def emit_l0(u):
        xk, xl = _XQMAP[u]
        wk, wl = _W8MAP[u]
        zz[u] = psab.tile([128, 512], _f32, name="zz", tag="zz")
        for qi in range(2):
            nc.tensor.matmul(
                zz[u][:, 256 * qi:256 * qi + 256],
                w8ts[wk][:, 320 * wl + 160 * qi:320 * wl + 160 * qi + 128],
                xts[xk][:, 512 * xl + 256 * qi:512 * xl + 256 * qi + 256],
                start=True, stop=True,
            )

    def emit_sq(u):
        sq[u] = sqpool.tile([128, 512], _f16, name="sq", tag="sq")
        if u % 2 == 0:
            nc.scalar.activation(sq[u][:], zz[u][:], SQUARE, scale=inv_b)
        else:
            nc.vector._custom_dve(
                _SQ_OP, out=sq[u][:], in0=zz[u][:],
                s0=inv_b2, s1=0.0, imm2=0.0)
        del zz[u]

    def emit_reduce(u0):
        # units u0, u0+1 -> quads 2u0..2u0+3 covering all 4 col strips
        for q in range(4):
            j = 2 * u0 + q
            u = u0 + q // 2
            qi = q % 2
            jj4, hb, _slot = _quad_slot(j)
            wk, wl = _W8MAP[u]
            fk, fl = _WFMAP[u]
            xk, xl = _XQMAP[u]
            # the first writer of each (strip, half) region uses start=True
            # (overwrite) in place of a zero-memset of the l2 banks
            first = j % 32 < 4
            nc.tensor.matmul(
                l2[hb][32 * jj4:32 * jj4 + 32, 0:256],
                w8ts[wk][:, 320 * wl + 160 * qi + 128:320 * wl + 160 * qi + 160],
                sq[u][:, 256 * qi:256 * qi + 256],
                start=first, stop=False,
                tile_position=(0, 32 * jj4),
                skip_group_check=True,
            )
            nc.tensor.matmul(
                l2[hb][32 * jj4:32 * jj4 + 32, 0:256],
                wfts[fk][:, 64 * fl + 32 * qi:64 * fl + 32 * qi + 32],
                xts[xk][:, 512 * xl + 256 * qi:512 * xl + 256 * qi + 256],
                start=False, stop=False,
                tile_position=(0, 32 * jj4),
                skip_group_check=True,
            )
        del sq[u0], sq[u0 + 1]

    def emit_evac(hb):
        o2 = opool.tile([128, 256], _f32, name="o2", tag=f"o2{hb}")
        nc.scalar.mul(o2[:], l2[hb][:, 0:256], inv_sout)
        if b2sb is not None:
            nc.vector.tensor_add(o2[:], o2[:], b2sb[:, 256 * hb:256 * hb + 256])
        nc.sync.dma_start(out=out_d[:, 256 * hb:256 * hb + 256], in_=o2[:])

    for t in range(NUNIT + 3):
        if t < NUNIT:
            emit_l0(t)
        if 0 <= t - 1 < NUNIT:
            emit_sq(t - 1)
        if t >= 4 and (t - 4) % 2 == 0 and t - 4 < NUNIT:
            emit_reduce(t - 4)
        if t == 21:
            # quads 0..31 (units 0-15) all reduced by t=20 -> stream out
            # the first output half while the back half still computes.
            emit_evac(0)
    emit_evac(1)


def _lin_ok(x, W0, b0, W1, b1):
    """gelu(z1) ~= z1/2 only holds when |z1| << 1; estimate max|z1| on a
    small batch sample (tanh-gelu approx is fine for a magnitude check)."""
    if bool(np.any(b1)):
        return False
    xs = x[:8].astype(np.float32)
    z0 = np.einsum('bdm,dmh->bdh', xs, W0.astype(np.float32))
    if bool(np.any(b0)):
        z0 = z0 + b0[None].astype(np.float32)
    h0 = 0.5 * z0 * (1.0 + np.tanh(0.7978845608 * (z0 + 0.044715 * z0**3)))
    z1 = np.einsum('bdh,dho->bdo', h0, W1.astype(np.float32))
    return float(np.abs(z1).max()) < 0.005


def _quad_ok(x, W0, b0, W1, b1):
    """The quadratic-gelu path additionally needs |z0| << 1."""
    if bool(np.any(b0)) or bool(np.any(b1)):
        return False
    if not _lin_ok(x, W0, b0, W1, b1):
        return False
    xs = x[:8].astype(np.float32)
    z0 = np.einsum('bdm,dmh->bdh', xs, W0.astype(np.float32))
    return float(np.abs(z0).max()) < 0.15


def _prep_quad_host(x, W0, W1, W2):
    """Global (all-neuron) eigendecomposition of the per-neuron quadratic
    form + pow2 scale selection."""
    import ml_dtypes
    f8 = ml_dtypes.float8_e4m3fn
    GC = 0.3989422804014327
    W0d = W0.astype(np.float64)
    veff = 0.5 * np.einsum('dho,do->dh', W1.astype(np.float64),
                           W2[:, :, 0].astype(np.float64))
    weff = 0.5 * np.einsum('dmh,dh->dm', W0d, veff)
    Q = GC * np.einsum('dmh,dh,dnh->dmn', W0d, veff, W0d)
    lam, V = np.linalg.eigh(Q)
    g = np.sqrt(np.abs(lam))[:, None, :] * V        # [D, m, 32]
    sgn = np.sign(lam)
    mx = np.maximum(np.abs(g).max(axis=(1, 2)), 1e-30)
    gs = 2.0 ** np.clip(np.round(np.log2(0.25 / mx)), -40, 40)
    gq = (g * gs[:, None, None]).astype(f8)
    zs = np.einsum('bdm,dmk->bdk',
                   x[:16].astype(np.float16).astype(np.float64),
                   gq.astype(np.float64))
    zmax = float(np.abs(zs).max()) * 1.5
    Bq = 2.0 ** np.ceil(np.log2(max(zmax, 1e-6) / 16.0))
    gs_med = float(np.median(gs))
    S_out = 2.0 ** np.round(np.log2((gs_med / Bq) ** 2))
    comp = S_out * Bq * Bq / gs ** 2                # pow2 per neuron
    assert comp.max() <= 256.0 and comp.min() >= 2.0 ** -9, (
        "comp outside fp8 range", comp.min(), comp.max())
    sgnq = (sgn * comp[:, None]).astype(f8)
    wfq = (weff * S_out).astype(np.float16)
    assert np.abs(wfq).max() < 60000.0, "wf overflow"
    return gq, sgnq, wfq, Bq, S_out


def _pack_core_quad(x, gq, sgnq, wfq, b2, c, use_b2):
    import ml_dtypes
    sl = slice(ND * c, ND * (c + 1))
    xc = x[:, sl, :]                                   # [B, 256, 32]
    xp = xc.transpose(1, 2, 0).reshape(64, 128, B)
    xp = np.ascontiguousarray(
        xp.transpose(1, 0, 2)).reshape(128, 64 * B).astype(np.float16)
    gqc, sgc, wfc = gq[sl], sgnq[sl], wfq[sl]
    w8 = np.zeros((128, 64 * 160), ml_dtypes.float8_e4m3fn)
    wf = np.zeros((128, 64 * 32), np.float16)
    for j in range(64):
        jj4, hb, slot = _quad_slot(j)
        for q in range(4):
            n = 4 * j + q
            w8[32 * q:32 * q + 32,
               160 * j + 32 * q:160 * j + 32 * q + 32] = gqc[n]
            w8[32 * q:32 * q + 32, 160 * j + 128 + 4 * slot + q] = sgc[n]
            wf[32 * q:32 * q + 32, 32 * j + 4 * slot + q] = wfc[n]
    m = {"xp": xp, "w8": w8, "wf": wf}
    if use_b2:
        b2bc = np.zeros((128, 512), np.float32)
        b2row = b2[sl, 0].astype(np.float32)
        for j in range(64):
            jj4, hb, slot = _quad_slot(j)
            for q in range(4):
                b2bc[32 * jj4 + 4 * slot + q,
                     256 * hb:256 * hb + 256] = b2row[4 * j + q]
        m["b2bc"] = b2bc
    return m


def _unstitch_quad(o):
    """o [128,512]: out[32jj4+4slot+q, 256hb+t] = y[t, 128hb+16slot+4jj4+q]."""
    o5 = o.reshape(4, 8, 4, 2, 256)                    # [jj4, slot, q, hb, t]
    return np.ascontiguousarray(
        o5.transpose(4, 3, 1, 0, 2)).reshape(256, 256)


def _get_program_quad(use_b2, Bq, S_out):
    key = ("quad", use_b2, Bq, S_out,
           os.environ.get("K_NCORES"), os.environ.get("K_NREP"))
    if key not in _PROGRAM_CACHE:
        _PROGRAM_CACHE[key] = _build_program_quad(
            use_b2, 1.0 / Bq, 1.0 / (Bq * Bq), 1.0 / S_out)
    return _PROGRAM_CACHE[key]


def _make_plan(x, W0, b0, W1, b1, W2, b2):
    """Shared by kernel() and test.py: returns (nc, in_maps, post)."""
    ncores = int(os.environ.get("K_NCORES", NCORES))
    use_b0 = bool(np.any(b0))
    use_b1 = bool(np.any(b1))
    use_b2 = bool(np.any(b2))
    if _quad_ok(x, W0, b0, W1, b1):
        gq, sgnq, wfq, Bq, S_out = _prep_quad_host(x, W0, W1, W2)
        nc = _get_program_quad(use_b2, Bq, S_out)
        in_maps = [_pack_core_quad(x, gq, sgnq, wfq, b2, c, use_b2)
                   for c in range(ncores)]
        post = _unstitch_quad
    else:
        use_lin = _lin_ok(x, W0, b0, W1, b1)
        nc = _get_program(use_b0, use_b1, use_b2, use_lin)
        in_maps = [
            _prep_core(x, W0, b0, W1, b1, W2, b2, c, use_b0, use_b1, use_b2,
                       use_lin)
            for c in range(ncores)
        ]
        post = _unstitch
    return nc, in_maps, post


def _prep_core(x, W0, b0, W1, b1, W2, b2, c, use_b0, use_b1, use_b2=False,
               use_lin=False):
    sl = slice(ND * c, ND * (c + 1))
    # xp[32q+m, 256j+t] = x[t, 4j+q, m]
    xc = x[:, sl, :]                                   # [B, 256, 32]
    xp = xc.transpose(1, 2, 0).reshape(64, 128, B)     # [j, 32q+m, t]
    xp = np.ascontiguousarray(
        xp.transpose(1, 0, 2)).reshape(128, 64 * B).astype(np.float16)
    # packed per-unit weights
    ucols = 384 if use_lin else 640
    wall = np.zeros((128, NUNIT * ucols), np.float16)
    W0c = W0[sl].astype(np.float16)                    # [256, 32, 64]
    if use_lin:
        # veff[d] = S_V * (W1[d] @ W2[d]) / 2  -- folds L1+gelu1+L2
        vc = (S_V * 0.5 * np.einsum(
            'dho,do->dh', W1[sl].astype(np.float64),
            W2[sl, :, 0].astype(np.float64))).astype(np.float16)  # [256, 64]
    else:
        W1c = W1[sl].astype(np.float16)                # [256, 64, 64]
        w2c = W2[sl, :, 0].astype(np.float16)          # [256, 64]
    for u in range(NUNIT):
        base = ucols * u
        for jj in range(2):                            # stack j = 2u+jj
            j = 2 * u + jj
            for a in range(2):
                for b in range(2):
                    r = 64 * a + 32 * b
                    cc = base + 128 * jj + 64 * b
                    wall[r:r + 32, cc:cc + 64] = W0c[4 * j + 2 * a + b]
        for c in range(4):
            p = 4 * u + c
            _, _, m_ = _l2slot(p)
            if use_lin:
                for e in range(2):
                    wall[64 * e:64 * e + 64,
                         base + 256 + 32 * c + 2 * m_ + e] = vc[2 * p + e]
            else:
                for b in range(2):
                    wall[64 * b:64 * b + 64,
                         base + 256 + 64 * c:base + 320 + 64 * c] = (
                        W1c[2 * p + b])
                for e in range(2):
                    wall[64 * e:64 * e + 64,
                         base + 512 + 32 * c + 2 * m_ + e] = w2c[2 * p + e]
    m = {"xp": xp, "wall": wall}
    if use_b2:
        # b2bc[32j+2m+e, 256hb+t] = b2[16m+8hb+2j+e]
        b2bc = np.zeros((128, 512), np.float32)
        b2row = b2[sl, 0].astype(np.float32)
        for p in range(NPAIR):
            j, hb, m_ = _l2slot(p)
            for e in range(2):
                b2bc[32 * j + 2 * m_ + e, 256 * hb:256 * hb + 256] = (
                    b2row[2 * p + e])
        m["b2bc"] = b2bc
    if use_b0:
        b0p = b0[sl].reshape(NPAIR, 2, H).transpose(1, 2, 0)
        m["b0p"] = np.ascontiguousarray(b0p).reshape(128, NPAIR).astype(np.float32)
    if use_b1:
        b1p = b1[sl].reshape(NPAIR, 2, H).transpose(1, 2, 0)
        m["b1p"] = np.ascontiguousarray(b1p).reshape(128, NPAIR).astype(np.float32)
    return m


def _unstitch(o):
    """o [128,512]: out[32j+2m+e, 256hb+t] = y[t, 16m+8hb+2j+e]."""
    o5 = o.reshape(4, 16, 2, 2, 256)                   # [j, m, e, hb, t]
    return np.ascontiguousarray(
        o5.transpose(4, 1, 3, 0, 2)).reshape(256, 256)  # [t, m,hb,j,e]


def kernel(pre_activation_history, W0, b0, W1, b1, W2, b2):
    x = np.asarray(pre_activation_history, np.float32)
    W0 = np.asarray(W0, np.float32)
    b0 = np.asarray(b0, np.float32)
    W1 = np.asarray(W1, np.float32)
    b1 = np.asarray(b1, np.float32)
    W2 = np.asarray(W2, np.float32)
    b2 = np.asarray(b2, np.float32)

    nc, in_maps, post = _make_plan(x, W0, b0, W1, b1, W2, b2)
    ncores = int(os.environ.get("K_NCORES", NCORES))
    res = run_bass_kernel_spmd(nc, in_maps, list(range(ncores)))
    y = np.zeros((B, D), np.float32)
    for c in range(ncores):
        y[:, ND * c:ND * (c + 1)] = post(res.results[c]["out"])
    return y



# revision 31
# speedup vs baseline: 1.0357x; 1.0177x over previous
"""Trainium2 Bass kernel for per-neuron MLPs (dense_mlp).

reference: out[b,d] = W2[d]^T.gelu(W1[d]^T.gelu(W0[d]^T.x[b,d,:]+b0)+b1)+b2
Shapes: x [256,2048,32], W0 [2048,32,64], W1 [2048,64,64], W2 [2048,64,1].

Sharding: D split across 8 cores (256 neurons each, fully independent).

Quadratic fast path (gated by _quad_ok): for this problem both hidden
pre-activations are tiny (|z0| < 0.1, |z1| < 5e-3), so
  gelu(z1) ~= z1/2          (collapses L1+gelu1+L2 into veff = W1@W2/2)
  gelu(z0) ~= z0/2 + c z0^2 (c = 1/sqrt(2pi); quartic term ~1e-5 rel)
and each neuron's whole MLP becomes
  out_d(x) = weff_d.x + sum_k s_k (g_k.x)^2
where Q_d = c W0 diag(veff) W0^T (32x32) = V diag(lam) V^T (host eigh),
g_k = sqrt|lam_k| v_k (fp8, per-neuron pow2 scale), s_k = sign(lam_k)
(carried as +-pow2 compensation in fp8), weff = W0.veff/2 (fp16).
This halves PE projections (32/neuron, not 64) and replaces the gelu LUT
stage with one cheap square per PSUM bank.  End-to-end rel err ~1.4e-3
vs the 2e-2 gate; the older lin/full pipelines remain as fallbacks.

Per-core dataflow (unit = 8 neurons = 2 quads, software-pipelined:
step t emits proj(t) | square(t-1) | reduce(t-4, pairs of units)):
  DMA: 17 chunks on BOTH HWDGE queues in consumption order - early-x on
      Scalar's queue, weights + late-x need-ordered on Sync's.  5.75MB
      total per core (x fp16 4MB, g/sgn fp8 1.25MB, weff fp16 0.5MB).
  proj: per quad one full-array matmul: block-diag g lhsT [128,128] fp8
      (rows 32q+m, cols 32q+k) x x-quad-stack [128,256] fp16 -> one PSUM
      bank zz [128,512] per unit (two quads side by side).
  square: whole units alternate ScalarE (Square LUT, scale 1/B) and DVE
      (custom C0*u^2 op) -> sq [128,512] fp16; one op per unit because
      the ~400ns fixed PSUM-access overhead dominates op size.
  reduce: per quad TWO 32-col-strip matmuls at tile_position (0,32(j%4))
      accumulate into l2[j//32] [128,512]: sgn strip (+-comp pow2) x sq
      gives the quadratic term; weff strip x the same x tile gives the
      linear term.  First writer per strip uses start=True (no memset).
      Batches of 4 quads keep all 4 column strips concurrently busy.
  evac: half 0 (quads 0-31) streams out at t=21 overlapping compute;
      half 1 after the loop.  o2 = l2 * (1/S_out) on ScalarE (+b2).
  Host re-stitches out[32(j%4)+4slot+q, 256hb+t] -> y[B, ND].
"""

import os
import sys

for _p in ("/opt/trn_rl_repo",):
    if _p not in sys.path:
        sys.path.insert(0, _p)

import numpy as np

import concourse.dve_ops as _dvo
from concourse import bacc, mybir, tile
from concourse import bass_utils as _bu
from concourse.bass_utils import run_bass_kernel_spmd


from concourse.dve_ops import DveOp, DveOpSpec, has_src1, lower as _dve_lower
from concourse.dve_spec import Spec, Src0, C0, C1, C2, One, sq

B = 256
D = 2048
M = 32
H = 64
NCORES = 8
ND = D // NCORES          # neurons per core = 256
NPAIR = ND // 2           # 128
NUNIT = ND // 8           # 32 units of 8 neurons (4 pairs)
GELU_C = 0.3989422804014327  # 1/sqrt(2*pi)
S_H1 = float(2 ** 14)     # fp16 scale for h1 (values ~1e-4 -> ~1.6)
S_V = float(2 ** 9)       # fp16 scale for veff = W1@W2/2 (values ~3e-5)

_f32 = mybir.dt.float32
_f16 = mybir.dt.float16


def _zc(c):
    """z0/h0 column of pair-in-unit c; concurrent row groups (c%2) get
    different PSUM banks."""
    return 512 * (c % 2) + 256 * (c // 2)


def _l2slot(p):
    """pair p -> (strip j, col half hb, partition slot m) in l2ps."""
    return p % 4, (p // 4) % 2, p // 8


_CH = [(0, 1), (1, 1), (2, 2), (4, 4), (8, 8), (16, 8), (24, 8)]
X_CHUNKS = list(_CH)
W_CHUNKS = list(_CH)


def _chunk_map(chunks):
    m = {}
    for k, (s, L) in enumerate(chunks):
        for u in range(s, s + L):
            m[u] = (k, u - s)
    return m


_XMAP = _chunk_map(X_CHUNKS)
_WMAP = _chunk_map(W_CHUNKS)


def _register_gelu_op():
    """out = u*(C1 + u*C0*(1 + u^2*C2)); with C0=S*c, C1=S/2, C2=-1/6 this is
    S*gelu(u) up to O(u^6) of the exact erf-gelu Taylor series."""
    name = "GELU_SCALED_ANT"
    for op in _dvo.OPS:
        if op.name == name:
            return op
    u = Src0
    body = u * (C1 + u * C0 * (One + sq(u) * C2))
    spec = Spec(
        body=body,
        reference=lambda in0, s0, s1, imm2: in0
        * (s1 + in0 * s0 * (1.0 + (in0 * in0) * imm2)),
    )
    shas = {}
    op = DveOp(name, spec, subdim=False, uops_sha=shas)
    _dvo.OPS.append(op)
    _dvo.CUSTOM_DVE_SPECS[name] = spec
    _dvo._SUB_OPCODE_FOR_NAME[name] = _dvo._CUSTOM_DVE_ROW_BASE + len(_dvo.OPS) - 1
    for ver in ("v3", "v4"):
        tmp = DveOpSpec(
            name=name,
            opcode=_dvo.get_dve_sub_opcode(name),
            uops=_dve_lower(spec, ver=ver),
            rd1_en=has_src1(spec),
        )
        shas[ver] = tmp.sha(ver)
    return op


_GELU_OP = _register_gelu_op()


def _register_sq_op():
    """out = C0 * Src0^2 — scaled square for the quadratic-gelu path."""
    name = "SQSCALE_ANT"
    for op in _dvo.OPS:
        if op.name == name:
            return op
    body = sq(Src0) * C0
    spec = Spec(
        body=body,
        reference=lambda in0, s0, s1, imm2: in0 * in0 * s0,
    )
    shas = {}
    op = DveOp(name, spec, subdim=False, uops_sha=shas)
    _dvo.OPS.append(op)
    _dvo.CUSTOM_DVE_SPECS[name] = spec
    _dvo._SUB_OPCODE_FOR_NAME[name] = _dvo._CUSTOM_DVE_ROW_BASE + len(_dvo.OPS) - 1
    for ver in ("v3", "v4"):
        tmp = DveOpSpec(
            name=name,
            opcode=_dvo.get_dve_sub_opcode(name),
            uops=_dve_lower(spec, ver=ver),
            rd1_en=has_src1(spec),
        )
        shas[ver] = tmp.sha(ver)
    return op


_SQ_OP = _register_sq_op()

_PROGRAM_CACHE = {}


def _build_program(use_b0, use_b1, use_b2, use_lin=False):
    ncores = int(os.environ.get("K_NCORES", NCORES))
    nrep = int(os.environ.get("K_NREP", 1))
    nc = bacc.Bacc("TRN2", target_bir_lowering=False, debug=False,
                   num_devices=ncores)

    ucols = 384 if use_lin else 640
    # x pair-stacks: xp[32q+m, 256j+t] = x[t, 4j+q, m]
    xp_d = nc.declare_dram_parameter("xp", [128, 64 * 256], _f16,
                                     isOutput=False)
    # all weights packed per unit.
    # full path (640 cols/unit: w0 256 | w1 256 | w2 128):
    #   w0 block: [64a+32b+m, 128*(j-2u)+64b+h] = W0[4j+2a+b][m,h]
    #   w1 block: [64b+h, 64c+o] = W1[2(4u+c)+b][h,o]
    #   w2 block: zero-padded blockdiag [64e+h, 32c+2m+e] = W2[2(4u+c)+e][h]
    # linearized path (384 cols/unit: w0 256 | veff 128), where
    #   veff[d] = S_V * (W1[d] @ W2[d]) / 2 replaces w1/w2 blocks.
    wall_d = nc.declare_dram_parameter("wall", [128, NUNIT * ucols], _f16,
                                       isOutput=False)
    if use_b2:
        b2_d = nc.declare_dram_parameter("b2bc", [128, 512], _f32,
                                         isOutput=False)
    if use_b0:
        # b0p[64b+h, p] = b0[2p+b][h]
        b0_d = nc.declare_dram_parameter("b0p", [128, NPAIR], _f32,
                                         isOutput=False)
    if use_b1:
        b1_d = nc.declare_dram_parameter("b1p", [128, NPAIR], _f32,
                                         isOutput=False)
    # out[32j+2m+e, 256hb+t] = y[t, 16m+8hb+2j+e]
    out_d = nc.declare_dram_parameter("out", [128, 512], _f32, isOutput=True)

    GELU = mybir.ActivationFunctionType.Gelu

    with tile.TileContext(nc) as tc:
        with (
            tc.tile_pool(name="wpool", bufs=1) as wpool,
            tc.tile_pool(name="xpool", bufs=1) as xpool,
            tc.tile_pool(name="h0pool", bufs=3) as h0pool,
            tc.tile_pool(name="h1pool", bufs=3) as h1pool,
            tc.tile_pool(name="opool", bufs=1) as opool,
            tc.tile_pool(name="psab", bufs=3, space="PSUM") as psab,
            tc.tile_pool(name="ps2", bufs=1, space="PSUM") as ps2,
        ):
            # Geometric unit-granular chunks; x chunks issue on the Sync
            # HWDGE queue, weight chunks on the Scalar HWDGE queue so the
            # two streams transfer concurrently and each queue only pays
            # ~650ns issue cost per chunk (7 chunks/queue, not 33 on one).
            xts = []
            wts = []
            deferred = []

            for i in range(max(len(X_CHUNKS), len(W_CHUNKS))):
                if i < len(X_CHUNKS):
                    s, L = X_CHUNKS[i]
                    xt = xpool.tile([128, L * 512], _f16, name="xt",
                                    tag=f"xt{i}")
                    nc.sync.dma_start(out=xt[:], in_=xp_d[:, s * 512:(s + L) * 512])
                    xts.append(xt)
                if i < len(W_CHUNKS):
                    s, L = W_CHUNKS[i]
                    wt = wpool.tile([128, L * ucols], _f16, name="wt",
                                    tag=f"wt{i}")
                    nc.scalar.dma_start(out=wt[:],
                                        in_=wall_d[:, s * ucols:(s + L) * ucols])
                    wts.append(wt)
            b0sb = b1sb = b2sb = None
            if use_b2:
                b2sb = wpool.tile([128, 512], _f32, tag="b2sb")
                nc.sync.dma_start(out=b2sb[:], in_=b2_d[:])
            if use_b0:
                b0sb = wpool.tile([128, NPAIR], _f32, tag="b0sb")
                nc.sync.dma_start(out=b0sb[:], in_=b0_d[:])
            if use_b1:
                b1sb = wpool.tile([128, NPAIR], _f32, tag="b1sb")
                nc.sync.dma_start(out=b1sb[:], in_=b1_d[:])

            for _rep in range(nrep):
                if use_lin:
                    _emit_body_lin(nc, h0pool, opool, psab, ps2,
                                   out_d, xts, wts, b0sb, b2sb, GELU,
                                   deferred)
                else:
                    _emit_body(nc, h0pool, h1pool, opool, psab, ps2,
                               out_d, xts, wts, b0sb, b1sb, b2sb, GELU)

    nc.finalize()
    return nc


def _emit_body_lin(nc, h0pool, opool, psab, ps2,
                   out_d, xts, wts, b0sb, b2sb, GELU, deferred=()):
    """gelu(z1) ~= z1/2 for |z1| << 1, so L1+gelu1+L2 collapse into one
    per-neuron vector veff = W1 @ W2 / 2 applied to h0 with the same
    zero-padded block-diag accumulate as L2."""
    l2ps = ps2.tile([128, 512], _f32, tag="l2")
    nc.vector.memset(l2ps[:], 0.0)

    z0 = {}
    h0 = {}

    def emit_l0(u):
        xk, xl = _XMAP[u]
        wk, wl = _WMAP[u]
        xt = xts[xk]
        wt = wts[wk]
        z0[u] = psab.tile([128, 1024], _f32, name="z0", tag="zz")
        for c in range(4):
            a = c % 2
            wcol = 384 * wl + 128 * (c // 2)
            xcol = 512 * xl + 256 * (c // 2)
            nc.tensor.matmul(
                z0[u][:, _zc(c):_zc(c) + 256],
                wt[64 * a:64 * a + 64, wcol:wcol + 128],
                xt[64 * a:64 * a + 64, xcol:xcol + 256],
                start=True, stop=True,
                tile_position=(64 * a, 0),
            )

    def emit_gelu0(u):
        # split across engines: ScalarE bank A (cols 0-511), DVE custom
        # poly bank B (cols 512-1023) -- parallel PSUM access, balanced
        # ~720ns vs ~680ns.
        h0[u] = h0pool.tile([128, 1024], _f16, name="h0", tag="h0")
        if b0sb is not None:
            for c in range(4):            # correct fallback: all-ScalarE
                p = 4 * u + c
                nc.scalar.activation(
                    h0[u][:, _zc(c):_zc(c) + 256],
                    z0[u][:, _zc(c):_zc(c) + 256],
                    GELU, bias=b0sb[:, p:p + 1], scale=1.0)
        else:
            nc.scalar.activation(h0[u][:, 0:512], z0[u][:, 0:512], GELU)
            nc.vector._custom_dve(
                _GELU_OP, out=h0[u][:, 512:1024], in0=z0[u][:, 512:1024],
                s0=GELU_C, s1=0.5, imm2=-1.0 / 6.0)
        del z0[u]

    def emit_l12(u):
        wk, wl = _WMAP[u]
        for c in range(4):
            p = 4 * u + c
            j, hb, m_ = _l2slot(p)
            wcol = 384 * wl + 256 + 32 * c
            nc.tensor.matmul(
                l2ps[32 * j:32 * j + 32, 256 * hb:256 * hb + 256],
                wts[wk][:, wcol:wcol + 32],
                h0[u][:, _zc(c):_zc(c) + 256],
                start=False, stop=False,
                tile_position=(0, 32 * j),
                skip_group_check=True,
            )
        del h0[u]

    for t in range(NUNIT + 2):
        if t < NUNIT:
            emit_l0(t)
        if 0 <= t - 1 < NUNIT:
            emit_gelu0(t - 1)
            for g, tile_, dram, lo, hi in deferred:
                if g == t:
                    # tiny GpSimd write orders the DMA (WAW) behind
                    # pipeline progress, keeping its transfer out of the
                    # startup ramp's bandwidth window
                    nc.gpsimd.tensor_copy(tile_[0:1, 0:2],
                                          h0[t - 1][0:1, 0:2])
                    nc.sync.dma_start(out=tile_[:], in_=dram[:, lo:hi])
        if 0 <= t - 2 < NUNIT:
            emit_l12(t - 2)

    o2 = opool.tile([128, 512], _f32, tag="o2")
    nc.scalar.mul(o2[:], l2ps[:], 1.0 / S_V)
    if b2sb is not None:
        nc.vector.tensor_add(o2[:], o2[:], b2sb[:])
    nc.sync.dma_start(out=out_d[:], in_=o2[:])


def _emit_body(nc, h0pool, h1pool, opool, psab, ps2,
               out_d, xts, wts, b0sb, b1sb, b2sb, GELU):
    l2ps = ps2.tile([128, 512], _f32, tag="l2")
    # Data is zeroed up front so every L2 matmul can use start=False:
    # first-writer overwrite and accumulate both produce 0 + v.
    nc.vector.memset(l2ps[:], 0.0)

    z0 = {}
    h0 = {}
    h1 = {}

    def emit_l0(u):
        xk, xl = _XMAP[u]
        wk, wl = _WMAP[u]
        xt = xts[xk]
        wt = wts[wk]
        z0[u] = psab.tile([128, 1024], _f32, name="z0", tag="zz")
        for c in range(4):
            a = c % 2
            wcol = 640 * wl + 128 * (c // 2)
            xcol = 512 * xl + 256 * (c // 2)
            nc.tensor.matmul(
                z0[u][:, _zc(c):_zc(c) + 256],
                wt[64 * a:64 * a + 64, wcol:wcol + 128],
                xt[64 * a:64 * a + 64, xcol:xcol + 256],
                start=True, stop=True,
                tile_position=(64 * a, 0),
            )

    def emit_gelu0(u):
        h0[u] = h0pool.tile([128, 1024], _f16, name="h0", tag="h0")
        if b0sb is not None:
            for c in range(4):
                p = 4 * u + c
                nc.scalar.activation(
                    h0[u][:, _zc(c):_zc(c) + 256],
                    z0[u][:, _zc(c):_zc(c) + 256],
                    GELU, bias=b0sb[:, p:p + 1], scale=1.0)
        else:
            nc.scalar.activation(h0[u][:], z0[u][:], GELU)
        del z0[u]

    def emit_l1_gelu1(u):
        z1 = psab.tile([128, 1024], _f32, name="z1", tag="zz")
        for c in range(4):
            p = 4 * u + c
            for b in range(2):
                rp = 64 * b
                wk, wl = _WMAP[u]
                nc.tensor.matmul(
                    z1[rp:rp + 64, 256 * c:256 * c + 256],
                    wts[wk][rp:rp + 64,
                            640 * wl + 256 + 64 * c:640 * wl + 320 + 64 * c],
                    h0[u][rp:rp + 64, _zc(c):_zc(c) + 256],
                    start=True, stop=True,
                    tile_position=(rp, rp),
                )
        gelu_in = z1
        if b1sb is not None:
            tmp = h0pool.tile([128, 1024], _f32, name="b1tmp", tag="b1tmp")
            for c in range(4):
                p = 4 * u + c
                nc.vector.tensor_scalar_add(
                    tmp[:, 256 * c:256 * c + 256],
                    z1[:, 256 * c:256 * c + 256],
                    b1sb[:, p:p + 1])
            gelu_in = tmp
        h1[u] = h1pool.tile([128, 1024], _f16, name="h1", tag="h1")
        nc.vector._custom_dve(
            _GELU_OP, out=h1[u][:], in0=gelu_in[:],
            s0=S_H1 * GELU_C, s1=S_H1 * 0.5, imm2=-1.0 / 6.0)
        del h0[u]

    def emit_l2(u):
        for c in range(4):
            p = 4 * u + c
            j, hb, m_ = _l2slot(p)
            ht = h1[u]
            wk, wl = _WMAP[u]
            wcol = 640 * wl + 512 + 32 * c
            nc.tensor.matmul(
                l2ps[32 * j:32 * j + 32, 256 * hb:256 * hb + 256],
                wts[wk][:, wcol:wcol + 32],
                ht[:, 256 * c:256 * c + 256],
                start=False, stop=False,
                tile_position=(0, 32 * j),
                skip_group_check=True,
            )
        del h1[u]

    for t in range(NUNIT + 3):
        if t < NUNIT:
            emit_l0(t)
        if 0 <= t - 1 < NUNIT:
            emit_gelu0(t - 1)
            emit_l1_gelu1(t - 1)
        if 0 <= t - 3 < NUNIT:
            emit_l2(t - 3)

    # ---- evac + store ----
    o2 = opool.tile([128, 512], _f32, tag="o2")
    nc.scalar.mul(o2[:], l2ps[:], 1.0 / S_H1)
    if b2sb is not None:
        nc.vector.tensor_add(o2[:], o2[:], b2sb[:])
    nc.sync.dma_start(out=out_d[:], in_=o2[:])


XQ_CHUNKS = [(0, 1), (1, 2), (3, 3), (6, 4), (10, 4), (14, 4), (18, 5),
             (23, 5), (28, 4)]
W8_CHUNKS = [(0, 1), (1, 4), (5, 8), (13, 9), (22, 10)]
WF_CHUNKS = [(0, 8), (8, 12), (20, 12)]
_XQMAP = _chunk_map(XQ_CHUNKS)
_W8MAP = _chunk_map(W8_CHUNKS)
_WFMAP = _chunk_map(WF_CHUNKS)
# x chunks 0-5 issue on the Scalar HWDGE queue; the rest on Sync in
# first-need order (wf chunk i is first needed ~4 units past its start)
_DMA_ORDER = [("w8", 0), ("x", 0), ("w8", 1), ("x", 1), ("x", 2),
              ("w8", 2), ("wf", 0), ("x", 3), ("x", 4), ("w8", 3),
              ("wf", 1), ("x", 5), ("x", 6), ("w8", 4), ("x", 7),
              ("wf", 2), ("x", 8)]
_f8 = mybir.dt.float8e4


def _quad_slot(j):
    """quad j -> (strip jj4, psum col half hb, 4-row slot)."""
    return j % 4, j // 32, (j % 32) // 4


def _build_program_quad(use_b2, inv_b, inv_b2, inv_sout):
    """gelu(z) = z/2 + c z^2 + O(z^4) for |z|<<1, so each neuron's MLP
    collapses to out = weff.x + sum_k s_k (g_k.x)^2 with g_k = sqrt|l_k| v_k
    from eigh of the 32x32 quadratic form.  32 projections (not 64) on the
    PE, cheap squares (not gelu LUT) on ScalarE/DVE alternating whole units,
    and per-quad reduce matmuls (sgn on squares + weff on x) accumulate
    straight into the output PSUM."""
    ncores = int(os.environ.get("K_NCORES", NCORES))
    nc = bacc.Bacc("TRN2", target_bir_lowering=False, debug=False,
                   num_devices=ncores)

    # xp[32q+m, 256j+t] = x[t, 4j+q, m]  (fp16, quad-stacked)
    xp_d = nc.declare_dram_parameter("xp", [128, 64 * 256], _f16,
                                     isOutput=False)
    # per quad j (160 cols): g block [32q+m, 32q+k] (block-diag, fp8,
    # per-neuron pow2 scale) | sgn strip [32q+k, 4*slot+q] = +-comp_n
    w8_d = nc.declare_dram_parameter("w8", [128, 64 * 160], _f8,
                                     isOutput=False)
    # wf strip per quad (32 cols): [32q+m, 4*slot+q] = S_out * weff_n[m]
    wf_d = nc.declare_dram_parameter("wf", [128, 64 * 32], _f16,
                                     isOutput=False)
    if use_b2:
        b2_d = nc.declare_dram_parameter("b2bc", [128, 512], _f32,
                                         isOutput=False)
    # out[32jj4+4slot+q, 256hb+t] = y[t, 4j+q]
    out_d = nc.declare_dram_parameter("out", [128, 512], _f32, isOutput=True)

    with tile.TileContext(nc) as tc:
        with (
            tc.tile_pool(name="wpool", bufs=1) as wpool,
            tc.tile_pool(name="xpool", bufs=1) as xpool,
            tc.tile_pool(name="sqpool", bufs=6) as sqpool,
            tc.tile_pool(name="opool", bufs=1) as opool,
            tc.tile_pool(name="psab", bufs=4, space="PSUM") as psab,
            tc.tile_pool(name="psl2", bufs=1, space="PSUM") as psl2,
        ):
            # Input DMAs ride the two HWDGE queues: the 6 early-x chunks on
            # Scalar (done issuing before the squares ramp up), everything
            # else need-ordered on Sync.  Per-queue transfers are FIFO in
            # consumption order; two queues double the issue rate, which
            # bounds the startup ramp.
            xts = [None] * len(XQ_CHUNKS)
            w8ts = [None] * len(W8_CHUNKS)
            wfts = [None] * len(WF_CHUNKS)

            def _issue(kind, i):
                if kind == "x":
                    s, L = XQ_CHUNKS[i]
                    t_ = xpool.tile([128, L * 512], _f16, name="xt",
                                    tag=f"xt{i}")
                    eng = nc.scalar if i < 6 else nc.sync
                    eng.dma_start(out=t_[:],
                                  in_=xp_d[:, s * 512:(s + L) * 512])
                    xts[i] = t_
                elif kind == "w8":
                    s, L = W8_CHUNKS[i]
                    t_ = wpool.tile([128, L * 320], _f8, name="w8t",
                                    tag=f"w8t{i}")
                    nc.sync.dma_start(out=t_[:],
                                      in_=w8_d[:, s * 320:(s + L) * 320])
                    w8ts[i] = t_
                else:
                    s, L = WF_CHUNKS[i]
                    t_ = wpool.tile([128, L * 64], _f16, name="wft",
                                    tag=f"wft{i}")
                    nc.sync.dma_start(out=t_[:],
                                      in_=wf_d[:, s * 64:(s + L) * 64])
                    wfts[i] = t_

            for kind, i in _DMA_ORDER:
                _issue(kind, i)
            b2sb = None
            if use_b2:
                b2sb = wpool.tile([128, 512], _f32, tag="b2sb")
                nc.sync.dma_start(out=b2sb[:], in_=b2_d[:])

            _emit_body_quad(nc, sqpool, opool, psab, psl2, out_d,
                            xts, w8ts, wfts, b2sb, inv_b, inv_b2, inv_sout)

    nc.finalize()
    return nc


def _emit_body_quad(nc, sqpool, opool, psab, psl2, out_d,
                    xts, w8ts, wfts, b2sb, inv_b, inv_b2, inv_sout):
    SQUARE = mybir.ActivationFunctionType.Square
    l2 = [psl2.tile([128, 512], _f32, name="l2", tag=f"l2{h}")
          for h in (0, 1)]

    zz = {}
    sq = {}

    def emit_l0(u):
        xk, xl = _XQMAP[u]
        wk, wl = _W8MAP[u]
        zz[u] = psab.tile([128, 512], _f32, name="zz", tag="zz")
        for qi in range(2):
            nc.tensor.matmul(
                zz[u][:, 256 * qi:256 * qi + 256],
                w8ts[wk][:, 320 * wl + 160 * qi:320 * wl + 160 * qi + 128],
                xts[xk][:, 512 * xl + 256 * qi:512 * xl + 256 * qi + 256],
                start=True, stop=True,
            )

    def emit_sq(u):
        sq[u] = sqpool.tile([128, 512], _f16, name="sq", tag="sq")
        if u % 2 == 0:
            nc.scalar.activation(sq[u][:], zz[u][:], SQUARE, scale=inv_b)
        else:
            nc.vector._custom_dve(
                _SQ_OP, out=sq[u][:], in0=zz[u][:],
                s0=inv_b2, s1=0.0, imm2=0.0)
        del zz[u]

    def emit_reduce(u0):
        # units u0, u0+1 -> quads 2u0..2u0+3 covering all 4 col strips
        for q in range(4):
            j = 2 * u0 + q
            u = u0 + q // 2
            qi = q % 2
            jj4, hb, _slot = _quad_slot(j)
            wk, wl = _W8MAP[u]
            fk, fl = _WFMAP[u]
            xk, xl = _XQMAP[u]
            # the first writer of each (strip, half) region uses start=True
            # (overwrite) in place of a zero-memset of the l2 banks
            first = j % 32 < 4
            nc.tensor.matmul(
                l2[hb][32 * jj4:32 * jj4 + 32, 0:256],
                w8ts[wk][:, 320 * wl + 160 * qi + 128:320 * wl + 160 * qi + 160],
                sq[u][:, 256 * qi:256 * qi + 256],
                start=first, stop=False,
                tile_position=(0, 32 * jj4),
                skip_group_check=True,
            )
            nc.tensor.matmul(
                l2[hb][32 * jj4:32 * jj4 + 32, 0:256],
                wfts[fk][:, 64 * fl + 32 * qi:64 * fl + 32 * qi + 32],
                xts[xk][:, 512 * xl + 256 * qi:512 * xl + 256 * qi + 256],
                start=False, stop=False,
                tile_position=(0, 32 * jj4),
                skip_group_check=True,
            )
        del sq[u0], sq[u0 + 1]

    def emit_evac(hb):
        o2 = opool.tile([128, 256], _f32, name="o2", tag=f"o2{hb}")
        nc.scalar.mul(o2[:], l2[hb][:, 0:256], inv_sout)
        if b2sb is not None:
            nc.vector.tensor_add(o2[:], o2[:], b2sb[:, 256 * hb:256 * hb + 256])
        nc.sync.dma_start(out=out_d[:, 256 * hb:256 * hb + 256], in_=o2[:])

    for t in range(NUNIT + 3):
        if t < NUNIT:
            emit_l0(t)
        if 0 <= t - 1 < NUNIT:
            emit_sq(t - 1)
        if t >= 4 and (t - 4) % 2 == 0 and t - 4 < NUNIT:
            emit_reduce(t - 4)
        if t == 19:
            # quads 0..31 (units 0-15) all reduced by t=18 -> stream out
            # the first output half while the back half still computes.
            emit_evac(0)
    emit_evac(1)


def _lin_ok(x, W0, b0, W1, b1):
    """gelu(z1) ~= z1/2 only holds when |z1| << 1; estimate max|z1| on a
    small batch sample (tanh-gelu approx is fine for a magnitude check)."""
    if bool(np.any(b1)):
        return False
    xs = x[:8].astype(np.float32)
    z0 = np.einsum('bdm,dmh->bdh', xs, W0.astype(np.float32))
    if bool(np.any(b0)):
        z0 = z0 + b0[None].astype(np.float32)
    h0 = 0.5 * z0 * (1.0 + np.tanh(0.7978845608 * (z0 + 0.044715 * z0**3)))
    z1 = np.einsum('bdh,dho->bdo', h0, W1.astype(np.float32))
    return float(np.abs(z1).max()) < 0.005


def _quad_ok(x, W0, b0, W1, b1):
    """The quadratic-gelu path additionally needs |z0| << 1."""
    if bool(np.any(b0)) or bool(np.any(b1)):
        return False
    if not _lin_ok(x, W0, b0, W1, b1):
        return False
    xs = x[:8].astype(np.float32)
    z0 = np.einsum('bdm,dmh->bdh', xs, W0.astype(np.float32))
    return float(np.abs(z0).max()) < 0.15


def _prep_quad_host(x, W0, W1, W2):
    """Global (all-neuron) eigendecomposition of the per-neuron quadratic
    form + pow2 scale selection."""
    import ml_dtypes
    f8 = ml_dtypes.float8_e4m3fn
    GC = 0.3989422804014327
    W0d = W0.astype(np.float64)
    veff = 0.5 * np.einsum('dho,do->dh', W1.astype(np.float64),
                           W2[:, :, 0].astype(np.float64))
    weff = 0.5 * np.einsum('dmh,dh->dm', W0d, veff)
    Q = GC * np.einsum('dmh,dh,dnh->dmn', W0d, veff, W0d)
    lam, V = np.linalg.eigh(Q)
    g = np.sqrt(np.abs(lam))[:, None, :] * V        # [D, m, 32]
    sgn = np.sign(lam)
    mx = np.maximum(np.abs(g).max(axis=(1, 2)), 1e-30)
    gs = 2.0 ** np.clip(np.round(np.log2(0.25 / mx)), -40, 40)
    gq = (g * gs[:, None, None]).astype(f8)
    zs = np.einsum('bdm,dmk->bdk',
                   x[:16].astype(np.float16).astype(np.float64),
                   gq.astype(np.float64))
    zmax = float(np.abs(zs).max()) * 1.5
    Bq = 2.0 ** np.ceil(np.log2(max(zmax, 1e-6) / 16.0))
    gs_med = float(np.median(gs))
    S_out = 2.0 ** np.round(np.log2((gs_med / Bq) ** 2))
    comp = S_out * Bq * Bq / gs ** 2                # pow2 per neuron
    assert comp.max() <= 256.0 and comp.min() >= 2.0 ** -9, (
        "comp outside fp8 range", comp.min(), comp.max())
    sgnq = (sgn * comp[:, None]).astype(f8)
    wfq = (weff * S_out).astype(np.float16)
    assert np.abs(wfq).max() < 60000.0, "wf overflow"
    return gq, sgnq, wfq, Bq, S_out


def _pack_core_quad(x, gq, sgnq, wfq, b2, c, use_b2):
    import ml_dtypes
    sl = slice(ND * c, ND * (c + 1))
    xc = x[:, sl, :]                                   # [B, 256, 32]
    xp = xc.transpose(1, 2, 0).reshape(64, 128, B)
    xp = np.ascontiguousarray(
        xp.transpose(1, 0, 2)).reshape(128, 64 * B).astype(np.float16)
    gqc, sgc, wfc = gq[sl], sgnq[sl], wfq[sl]
    w8 = np.zeros((128, 64 * 160), ml_dtypes.float8_e4m3fn)
    wf = np.zeros((128, 64 * 32), np.float16)
    for j in range(64):
        jj4, hb, slot = _quad_slot(j)
        for q in range(4):
            n = 4 * j + q
            w8[32 * q:32 * q + 32,
               160 * j + 32 * q:160 * j + 32 * q + 32] = gqc[n]
            w8[32 * q:32 * q + 32, 160 * j + 128 + 4 * slot + q] = sgc[n]
            wf[32 * q:32 * q + 32, 32 * j + 4 * slot + q] = wfc[n]
    m = {"xp": xp, "w8": w8, "wf": wf}
    if use_b2:
        b2bc = np.zeros((128, 512), np.float32)
        b2row = b2[sl, 0].astype(np.float32)
        for j in range(64):
            jj4, hb, slot = _quad_slot(j)
            for q in range(4):
                b2bc[32 * jj4 + 4 * slot + q,
                     256 * hb:256 * hb + 256] = b2row[4 * j + q]
        m["b2bc"] = b2bc
    return m


def _unstitch_quad(o):
    """o [128,512]: out[32jj4+4slot+q, 256hb+t] = y[t, 128hb+16slot+4jj4+q]."""
    o5 = o.reshape(4, 8, 4, 2, 256)                    # [jj4, slot, q, hb, t]
    return np.ascontiguousarray(
        o5.transpose(4, 3, 1, 0, 2)).reshape(256, 256)


def _get_program_quad(use_b2, Bq, S_out):
    key = ("quad", use_b2, Bq, S_out,
           os.environ.get("K_NCORES"), os.environ.get("K_NREP"))
    if key not in _PROGRAM_CACHE:
        _PROGRAM_CACHE[key] = _build_program_quad(
            use_b2, 1.0 / Bq, 1.0 / (Bq * Bq), 1.0 / S_out)
    return _PROGRAM_CACHE[key]


def _make_plan(x, W0, b0, W1, b1, W2, b2):
    """Shared by kernel() and test.py: returns (nc, in_maps, post)."""
    ncores = int(os.environ.get("K_NCORES", NCORES))
    use_b0 = bool(np.any(b0))
    use_b1 = bool(np.any(b1))
    use_b2 = bool(np.any(b2))
    if _quad_ok(x, W0, b0, W1, b1):
        gq, sgnq, wfq, Bq, S_out = _prep_quad_host(x, W0, W1, W2)
        nc = _get_program_quad(use_b2, Bq, S_out)
        in_maps = [_pack_core_quad(x, gq, sgnq, wfq, b2, c, use_b2)
                   for c in range(ncores)]
        post = _unstitch_quad
    else:
        use_lin = _lin_ok(x, W0, b0, W1, b1)
        nc = _get_program(use_b0, use_b1, use_b2, use_lin)
        in_maps = [
            _prep_core(x, W0, b0, W1, b1, W2, b2, c, use_b0, use_b1, use_b2,
                       use_lin)
            for c in range(ncores)
        ]
        post = _unstitch
    return nc, in_maps, post


def _prep_core(x, W0, b0, W1, b1, W2, b2, c, use_b0, use_b1, use_b2=False,
               use_lin=False):
    sl = slice(ND * c, ND * (c + 1))
    # xp[32q+m, 256j+t] = x[t, 4j+q, m]
    xc = x[:, sl, :]                                   # [B, 256, 32]
    xp = xc.transpose(1, 2, 0).reshape(64, 128, B)     # [j, 32q+m, t]
    xp = np.ascontiguousarray(
        xp.transpose(1, 0, 2)).reshape(128, 64 * B).astype(np.float16)
    # packed per-unit weights
    ucols = 384 if use_lin else 640
    wall = np.zeros((128, NUNIT * ucols), np.float16)
    W0c = W0[sl].astype(np.float16)                    # [256, 32, 64]
    if use_lin:
        # veff[d] = S_V * (W1[d] @ W2[d]) / 2  -- folds L1+gelu1+L2
        vc = (S_V * 0.5 * np.einsum(
            'dho,do->dh', W1[sl].astype(np.float64),
            W2[sl, :, 0].astype(np.float64))).astype(np.float16)  # [256, 64]
    else:
        W1c = W1[sl].astype(np.float16)                # [256, 64, 64]
        w2c = W2[sl, :, 0].astype(np.float16)          # [256, 64]
    for u in range(NUNIT):
        base = ucols * u
        for jj in range(2):                            # stack j = 2u+jj
            j = 2 * u + jj
            for a in range(2):
                for b in range(2):
                    r = 64 * a + 32 * b
                    cc = base + 128 * jj + 64 * b
                    wall[r:r + 32, cc:cc + 64] = W0c[4 * j + 2 * a + b]
        for c in range(4):
            p = 4 * u + c
            _, _, m_ = _l2slot(p)
            if use_lin:
                for e in range(2):
                    wall[64 * e:64 * e + 64,
                         base + 256 + 32 * c + 2 * m_ + e] = vc[2 * p + e]
            else:
                for b in range(2):
                    wall[64 * b:64 * b + 64,
                         base + 256 + 64 * c:base + 320 + 64 * c] = (
                        W1c[2 * p + b])
                for e in range(2):
                    wall[64 * e:64 * e + 64,
                         base + 512 + 32 * c + 2 * m_ + e] = w2c[2 * p + e]
    m = {"xp": xp, "wall": wall}
    if use_b2:
        # b2bc[32j+2m+e, 256hb+t] = b2[16m+8hb+2j+e]
        b2bc = np.zeros((128, 512), np.float32)
        b2row = b2[sl, 0].astype(np.float32)
        for p in range(NPAIR):
            j, hb, m_ = _l2slot(p)
            for e in range(2):
                b2bc[32 * j + 2 * m_ + e, 256 * hb:256 * hb + 256] = (
                    b2row[2 * p + e])
        m["b2bc"] = b2bc
    if use_b0:
        b0p = b0[sl].reshape(NPAIR, 2, H).transpose(1, 2, 0)
        m["b0p"] = np.ascontiguousarray(b0p).reshape(128, NPAIR).astype(np.float32)
    if use_b1:
        b1p = b1[sl].reshape(NPAIR, 2, H).transpose(1, 2, 0)
        m["b1p"] = np.ascontiguousarray(b1p).reshape(128, NPAIR).astype(np.float32)
    return m


def _unstitch(o):
    """o [128,512]: out[32j+2m+e, 256hb+t] = y[t, 16m+8hb+2j+e]."""
    o5 = o.reshape(4, 16, 2, 2, 256)                   # [j, m, e, hb, t]
    return np.ascontiguousarray(
        o5.transpose(4, 1, 3, 0, 2)).reshape(256, 256)  # [t, m,hb,j,e]


def kernel(pre_activation_history, W0, b0, W1, b1, W2, b2):
    x = np.asarray(pre_activation_history, np.float32)
    W0 = np.asarray(W0, np.float32)
    b0 = np.asarray(b0, np.float32)
    W1 = np.asarray(W1, np.float32)
    b1 = np.asarray(b1, np.float32)
    W2 = np.asarray(W2, np.float32)
    b2 = np.asarray(b2, np.float32)

    nc, in_maps, post = _make_plan(x, W0, b0, W1, b1, W2, b2)
    ncores = int(os.environ.get("K_NCORES", NCORES))
    res = run_bass_kernel_spmd(nc, in_maps, list(range(ncores)))
    y = np.zeros((B, D), np.float32)
    for c in range(ncores):
        y[:, ND * c:ND * (c + 1)] = post(res.results[c]["out"])
    return y

